# revision 28
# baseline (speedup 1.0000x reference)
"""Multi-head causal attention with RoPE on 8 Trainium2 NeuronCores.

Sharding: data-parallel over batch (B=2) x tensor-parallel over heads
(16 heads -> 4 groups of 4). Core c handles batch c//4, heads
[(c%4)*4, (c%4)*4+4). Each core computes a partial y = attn_out @ W_o
for its head group; the host sums the 4 partials per batch (the "W_o
all-reduce").

v4 design (v2 dtypes + early-start scheduling):
  - Q/K projections in fp8 DoubleRow from an fp8 x^T copy, with the
    fp8 weights pre-scaled x16 (escapes fp8e4m3 subnormals; the exp
    scale absorbs 1/256). The value path (V, probs, o, W_o) stays bf16:
    fp8 noise in a dot-product operand does NOT average down, and
    value-path fp8 measured ~6% output error.
  - Junk warm-up matmuls at t=0 hold the PE HAM clock gate open through
    the input-DMA window, and the prologue is restructured to unblock
    the first exp at ~14us (vs ~51us): K/Q mb0 chunk-major over the
    arriving fp8 x chunks -> rope half0 immediately; V mb0 chunk-major
    over the arriving bf16 x chunks; only transposes sb0-7 run before
    attention (sb8-15 + all of mb1 are fillers).
  - V-block transposes for the filler phase go through the DMA xbar
    (dma_start_transpose) instead of TensorE, freeing ~6us of PE time
    in the attention phase where PE is the co-bottleneck with ACT.
  - Attention: one head per pass, q in 1024-col passes, scores^T[t,q]
    as a single matmul per t-block, exp on ACT (the pass pacer), PV
    software-pipelined one iteration behind, denominator as a 65th
    ones-column of V. Causal mask preloaded additively (-1e7) via an
    identity-transpose matmul so the exp->PV path stays clean.
  - Epilogue per 512-col PSUM bank: den copy -> reciprocal_approx_fast
    -> gpsimd partition_broadcast -> one DVE multiply from PSUM.
  - y (o @ W_o) quanta interleave as fillers: sb0-7 during the last
    head's second pass, sb8-11 appended mid-pass once that pass's
    bank-A epilogue lands, so the tail only drains sb12-15.
"""

import os
import sys
from collections import deque
from contextlib import ExitStack

import numpy as np

for _p in ("/opt/trn_rl_repo",):
    if os.path.isdir(_p) and _p not in sys.path:
        sys.path.insert(0, _p)

import ml_dtypes  # noqa: E402

BF16 = ml_dtypes.bfloat16
F8 = ml_dtypes.float8_e4m3fn

B, S, E = 2, 2048, 1024
H, DH = 16, 64
NCORES = 8
HPC = H // 4          # 4 heads per core
DC = HPC * DH         # 256 head dims per core
ATTN_SCALE = 1.0 / 32.0  # 1/sqrt(E)
ROPE_BASE = 10000.0
P = 128
NSB = S // P          # 16 sequence blocks
NEC = E // P          # 8 E chunks
MB = DC // P          # 2 partition blocks of head dims

QK_SCALE = 16.0       # host pre-scale on W_q/W_k (fp8 subnormal escape)
EXP_SCALE = ATTN_SCALE / (QK_SCALE * QK_SCALE)
MASK_VAL = -1e7       # additive causal mask; * EXP_SCALE -> exp underflows to 0

DMA_TP = os.environ.get("KNL_DMATP", "1") == "1"

_PROG = None


def _perm64():
    """perm[j] = original head-dim index stored at permuted position j.

    Quadrant q of the permuted layout holds RoPE pairs i in
    [16q, 16q+16): even elements (2i) at slots 0-15, odd (2i+1) at
    slots 16-31. The rotation partner is then always +-16 partitions
    away within one 32-partition quadrant (stream_shuffle range).
    """
    j = np.arange(64)
    qd, r = j // 32, j % 32
    i = 16 * qd + (r % 16)
    return 2 * i + (r >= 16)


def _cos_sin_tiles():
    pl = np.arange(P) % 64
    qd, r = pl // 32, pl % 32
    i = 16 * qd + (r % 16)
    inv = ROPE_BASE ** (-(2.0 * i) / DH)
    ang = np.arange(S)[None, :] * inv[:, None]          # (128, S)
    sgn = np.where(r < 16, -1.0, 1.0)[:, None]
    return ang, sgn


def _build_program(debug=False):
    import concourse.bacc as bacc
    import concourse.tile as tile
    from concourse import masks, mybir

    f32 = mybir.dt.float32
    bf16 = mybir.dt.bfloat16
    f8 = mybir.dt.float8e4
    AF = mybir.ActivationFunctionType
    DR = mybir.MatmulPerfMode.DoubleRow

    nc = bacc.Bacc("TRN2", target_bir_lowering=False, debug=False)
    xbt = nc.dram_tensor("xbt", [E, S], bf16, kind="ExternalInput").ap()
    # fp8 copies of x^T / W_q / W_k in DoubleRow pair layout
    # [p, chunk-pair i, j, *]: element (2i+j)*128+p of the E axis
    xbt8 = nc.dram_tensor("xbt8", [P, NEC // 2, 2, S], f8,
                          kind="ExternalInput").ap()
    wq8 = nc.dram_tensor("wq8", [P, NEC // 2, 2, DC], f8,
                         kind="ExternalInput").ap()
    wk8 = nc.dram_tensor("wk8", [P, NEC // 2, 2, DC], f8,
                         kind="ExternalInput").ap()
    wv = nc.dram_tensor("wv", [E, DC], bf16, kind="ExternalInput").ap()
    wo = nc.dram_tensor("wo", [DC, E], bf16, kind="ExternalInput").ap()
    cosr = nc.dram_tensor("cosr", [P, S], bf16, kind="ExternalInput").ap()
    sinr = nc.dram_tensor("sinr", [P, S], bf16, kind="ExternalInput").ap()
    cmask = nc.dram_tensor("cmask", [P, P], bf16, kind="ExternalInput").ap()
    y = nc.dram_tensor("y", [S, E], bf16, kind="ExternalOutput").ap()
    # tiny output read from the warm-up accumulator so neuronxcc cannot
    # dead-code-eliminate the junk matmuls that keep the HAM clock warm
    warm = nc.dram_tensor("warm", [1, 4], f32, kind="ExternalOutput").ap()
    dbg = {}
    if debug:
        for nm, shp in (
            ("dkT", [P, MB, S]), ("dqz", [P, MB, 2, S]), ("dvT", [P, MB, S]),
            ("dvn", [P, NSB, HPC, 65]), ("donrm", [P, MB, S]),
        ):
            dbg[nm] = nc.dram_tensor(nm, shp, bf16, kind="ExternalOutput").ap()

    with ExitStack() as ctx:
        tc = ctx.enter_context(tile.TileContext(nc))
        consts = ctx.enter_context(tc.tile_pool(name="consts", bufs=1))
        persist = ctx.enter_context(tc.tile_pool(name="persist", bufs=1))

        ident = consts.tile([P, P], bf16, tag="ident")
        junk = consts.tile([P, 512], bf16, tag="junk")
        wk_t = consts.tile([P, NEC // 2, 2, DC], f8, tag="wk")
        wq_t = consts.tile([P, NEC // 2, 2, DC], f8, tag="wq")
        wv_t = consts.tile([P, NEC, DC], bf16, tag="wv")
        wo_t = consts.tile([P, MB, E], bf16, tag="wo")
        cos_t = consts.tile([P, S], bf16, tag="cos")
        sin_t = consts.tile([P, S], bf16, tag="sin")
        msk_t = consts.tile([P, P], bf16, tag="msk")

        xT = persist.tile([P, NEC, S], bf16, tag="xT")
        xT8 = persist.tile([P, NEC // 2, 2, S], f8, tag="xT8")
        kcT = persist.tile([P, MB, S], bf16, tag="kcT")
        qcT = persist.tile([P, MB, S], bf16, tag="qcT")
        vT = persist.tile([P, MB, S], bf16, tag="vT")
        # RoPE'd K^T, and Q^T zero-padded per head parity: slice
        # [:, mb, par, :] has head (2*mb+par)'s 64 rows live and the
        # other 64 rows zero, so scores use the FULL 128-row K^T block
        # as lhsT (the HAM clock gate never grants full clock to
        # partial-height matmul streams; zero rows contribute 0).
        kT = persist.tile([P, MB, S], bf16, tag="kT")
        qz = persist.tile([P, MB, 2, S], bf16, tag="qz")
        vn = persist.tile([P, NSB, HPC, 65], bf16, tag="vn")
        onrm = persist.tile([P, MB, S], bf16, tag="onrm")

        # ---- t=0 setup: memsets, masks, warm-up, DMA streams ----
        nc.vector.memset(junk[:], 1.0)
        nc.vector.memset(vn[:, :, :, 64:65], 1.0)
        nc.gpsimd.memset(qz[0:DH, :, 1, :], 0.0)
        nc.gpsimd.memset(qz[DH:P, :, 0, :], 0.0)
        masks.make_identity(nc, ident[:])

        wv_r = wv.rearrange("(c p) m -> p c m", p=P)
        # The rings sustain ~130 GB/s each, so ordering is everything:
        # x8 (gates the exp stream via K/Q+rope) goes first on both
        # rings, then the rope tables, then the 4MB bf16 x^T (gates only
        # the deferred-PV value path), then late weights.
        # sync ring: wk8, x8 evens, sin h0, wvA(mb0), xbt evens, sin h1, wvB
        # scalar ring: wq8, x8 odds, cos h0, cmask, xbt odds, cos h1, wo
        nc.sync.dma_start(wk_t[:], wk8)
        nc.scalar.dma_start(wq_t[:], wq8)
        for ec in range(NEC // 2):
            eng = nc.sync if ec % 2 == 0 else nc.scalar
            eng.dma_start(xT8[:, ec, :, :], xbt8[:, ec, :, :])
        nc.sync.dma_start(sin_t[:, 0:1024], sinr[:, 0:1024])
        nc.scalar.dma_start(cos_t[:, 0:1024], cosr[:, 0:1024])
        nc.scalar.dma_start(msk_t[:], cmask)
        nc.sync.dma_start(wv_t[:, :, 0:P], wv_r[:, :, 0:P])
        for ec in range(NEC):
            eng = nc.sync if ec % 2 == 0 else nc.scalar
            eng.dma_start(xT[:, ec, :], xbt[ec * P:(ec + 1) * P, :])
            if ec == 1:
                nc.sync.dma_start(sin_t[:, 1024:2048], sinr[:, 1024:2048])
                nc.scalar.dma_start(cos_t[:, 1024:2048], cosr[:, 1024:2048])
        nc.sync.dma_start(wv_t[:, :, P:DC], wv_r[:, :, P:DC])
        nc.scalar.dma_start(wo_t[:], wo.rearrange("(c p) n -> p c n", p=P))

        # exp spline table preload (one-time ~1.3us) off the critical path
        scr = consts.tile([P, 16], f32, tag="scr")
        nc.scalar.activation(scr[:], junk[:, 0:16], AF.Exp, scale=EXP_SCALE)
        # gpsimd ANT-lib preload: partition_broadcast's custom library
        # loads once here (~9us, hidden under input DMA)
        scr2 = consts.tile([16, 16], f32, tag="scr2")
        scr3 = consts.tile([1, 16], f32, tag="scr3")
        nc.vector.memset(scr3[:], 1.0)
        nc.gpsimd.partition_broadcast(scr2[:], scr3[:])

        # warm-up matmuls: the PE must stay busy from t=0 until the first
        # x8 chunk lands (~13us: ~6us queue bootstrap + transfer), else
        # the HAM MID window re-throttles the clock to 1.2 GHz for the
        # whole prologue. One ACCUMULATION CHAIN whose result is DMA'd
        # to a tiny output, so neuronxcc cannot dead-code-eliminate it
        # (independent dead-write matmuls get pruned).
        with ExitStack() as wctx:
            wu_ps = wctx.enter_context(
                tc.tile_pool(name="wu_ps", bufs=1, space="PSUM")
            )
            wu = wu_ps.tile([P, 512], f32, tag="wu")
            for r in range(24):
                nc.tensor.matmul(
                    wu[:], lhsT=junk[:, 0:P], rhs=junk[:],
                    start=(r == 0), stop=(r == 23),
                )
            ws = consts.tile([1, 4], f32, tag="ws")
            nc.vector.tensor_copy(ws[:], wu[0:1, 0:4])
            nc.sync.dma_start(warm, ws[:])

        # ---- K/Q mb0 chunk-major over arriving x8 chunks ----
        # ka and qa live in separate pools: qa's 4 banks are released
        # right after the mb0 casts (the burst-phase score buffers land
        # there), while ka's banks are reused for the K mb1 projection,
        # which fills the otherwise-idle PE window between the mb0
        # projections and the rope-gated first scores.
        kqA = ExitStack()
        kqA_ps = kqA.enter_context(tc.tile_pool(name="kqA_ps", bufs=1,
                                                space="PSUM"))
        with ExitStack() as kqB:
            kqB_ps = kqB.enter_context(
                tc.tile_pool(name="kqB_ps", bufs=1, space="PSUM")
            )
            ka = kqA_ps.tile([P, S], f32, tag="ka")
            qa = kqB_ps.tile([P, S], f32, tag="qa")
            for i in range(NEC // 2):
                for wt, acc in ((wk_t, ka), (wq_t, qa)):
                    for qt in range(4):
                        nc.tensor.matmul(
                            acc[:, qt * 512:(qt + 1) * 512],
                            lhsT=wt[:, i, :, 0:P],
                            rhs=xT8[:, i, :, qt * 512:(qt + 1) * 512],
                            perf_mode=DR,
                            start=(i == 0),
                            stop=(i == NEC // 2 - 1),
                        )
            # PSUM -> bf16 SBUF; DVE takes only kcT half0 (the rope
            # critical chain), ACT (idle during the prologue) the rest
            nc.scalar.copy(qcT[:, 0, 0:1024], qa[:, 0:1024])
            nc.vector.tensor_copy(kcT[:, 0, 0:1024], ka[:, 0:1024])
            nc.scalar.copy(kcT[:, 0, 1024:2048], ka[:, 1024:2048])
            nc.scalar.copy(qcT[:, 0, 1024:2048], qa[:, 1024:2048])

        shuf_mask = list(range(16, 32)) + list(range(16))
        sh_pool = ctx.enter_context(tc.tile_pool(name="sh", bufs=2))

        def rope_k_half(mb, hf):
            sl = slice(1024 * hf, 1024 * hf + 1024)
            sh = sh_pool.tile([P, 1024], bf16, tag="shk", name=f"shk{mb}_{hf}")
            nc.vector.stream_shuffle(sh[:], kcT[:, mb, sl], shuf_mask)
            nc.vector.tensor_mul(sh[:], sh[:], sin_t[:, sl])
            nc.vector.tensor_mul(kT[:, mb, sl], kcT[:, mb, sl], cos_t[:, sl])
            nc.vector.tensor_add(kT[:, mb, sl], kT[:, mb, sl], sh[:])

        def rope_q_half(mb, hf):
            sl = slice(1024 * hf, 1024 * hf + 1024)
            sh = sh_pool.tile([P, 1024], bf16, tag="shq", name=f"shq{mb}_{hf}")
            nc.vector.stream_shuffle(sh[:], qcT[:, mb, sl], shuf_mask)
            nc.vector.tensor_mul(sh[:], sh[:], sin_t[:, sl])
            qr = sh_pool.tile([P, 1024], bf16, tag="qr", name=f"qr{mb}_{hf}")
            nc.vector.tensor_mul(qr[:], qcT[:, mb, sl], cos_t[:, sl])
            nc.vector.tensor_add(qr[:], qr[:], sh[:])
            nc.vector.tensor_copy(qz[0:DH, mb, 0, sl], qr[0:DH, :])
            nc.vector.tensor_copy(qz[DH:P, mb, 1, sl], qr[DH:P, :])

        # unblock h0 pass0 ASAP: only mb0 half0 rope is on its path
        rope_k_half(0, 0)
        rope_q_half(0, 0)

        # ---- K mb1 in the pre-burst PE-idle window (xT8 is resident) ----
        ka1 = kqA_ps.tile([P, S], f32, tag="ka", name="ka1")
        for i in range(NEC // 2):
            for qt in range(4):
                nc.tensor.matmul(
                    ka1[:, qt * 512:(qt + 1) * 512],
                    lhsT=wk_t[:, i, :, P:DC],
                    rhs=xT8[:, i, :, qt * 512:(qt + 1) * 512],
                    perf_mode=DR,
                    start=(i == 0),
                    stop=(i == NEC // 2 - 1),
                )
        nc.vector.tensor_copy(kcT[:, 1, 0:1024], ka1[:, 0:1024])
        nc.vector.tensor_copy(kcT[:, 1, 1024:2048], ka1[:, 1024:2048])
        kqA.close()

        def vn_dst(sb, mb):
            return vn[:, sb, 2 * mb:2 * mb + 2, 0:64]

        # rope halves1 go on the DVE queue now, ahead of the V-phase DVE
        # work (qz half1 is needed when h0 pass1 starts); K mb1 rope
        # follows (it has until head 2)
        rope_q_half(0, 1)
        rope_k_half(0, 1)
        rope_k_half(1, 0)
        rope_k_half(1, 1)

        # ---- attention + filler machinery ----
        # Fillers are work quanta interleaved into the ACT-paced
        # attention iterations. CRITICAL: a filler that WRITES data read
        # by a later pass must be emitted (program order) before that
        # pass's reads -- Tile deps only point backward -- so the deque
        # has a hard flush point before h==2.
        fillers = deque()

        def emit_fillers(n):
            for _ in range(n):
                if not fillers:
                    return
                fillers.popleft()()

        def flush_fillers():
            while fillers:
                fillers.popleft()()

        attctx = ExitStack()
        sc_ps = attctx.enter_context(
            tc.tile_pool(name="sc_ps", bufs=2, space="PSUM")
        )
        # deep probs buffering: the first two passes run with PV fully
        # deferred (their V^T blocks arrive only at ~x^T-DMA completion),
        # so up to 16 pt tiles are alive at once
        ptp = ctx.enter_context(tc.tile_pool(name="ptp", bufs=18))
        dn = ctx.enter_context(tc.tile_pool(name="dn", bufs=2))
        pools = {}
        ycfg = {"pool": None, "tail": False}
        ys_pool = ctx.enter_context(tc.tile_pool(name="ys", bufs=4))
        vs_pool = ctx.enter_context(tc.tile_pool(name="vs", bufs=3))

        # --- filler generators (transposes, proj quarters, y halves) ---
        def mk_tp(sb, mb):
            if DMA_TP:
                def f():
                    # DMA-xbar transpose into contiguous staging, then a
                    # DVE copy into vn's 65-wide slot layout; zero PE.
                    eng = nc.sync if sb % 2 == 0 else nc.scalar
                    vs = vs_pool.tile([P, P], bf16, tag="vs",
                                      name=f"vs{mb}_{sb}")
                    eng.dma_start_transpose(
                        vs[:], vT[:, mb, sb * P:(sb + 1) * P]
                    )
                    nc.vector.tensor_copy(
                        vn_dst(sb, mb),
                        vs[:].rearrange("p (a b) -> p a b", a=2),
                    )
            else:
                def f():
                    tp = pools["yq"].tile([P, P], bf16, tag="pq",
                                          name=f"tp{mb}s_{sb}")
                    nc.tensor.transpose(
                        tp[:], vT[:, mb, sb * P:(sb + 1) * P], ident[:]
                    )
                    nc.vector.tensor_copy(
                        vn_dst(sb, mb), tp[:].rearrange("p (a b) -> p a b", a=2)
                    )
            return f

        pq_state = {}

        def make_proj_quarter(wt, dst, qi, tagname, fp8=False):
            steps = []
            nsteps = NEC // 2 if fp8 else NEC

            def mk_mm(i):
                def f():
                    if i == 0:
                        pq_state["t"] = pools["yq"].tile(
                            [P, 512], f32, tag="pq",
                            name=f"pq_{tagname}_{qi}",
                        )
                    if fp8:
                        nc.tensor.matmul(
                            pq_state["t"][:],
                            lhsT=wt[:, i, :, P:DC],
                            rhs=xT8[:, i, :, 512 * qi:512 * qi + 512],
                            perf_mode=DR,
                            start=(i == 0),
                            stop=(i == nsteps - 1),
                        )
                    else:
                        nc.tensor.matmul(
                            pq_state["t"][:],
                            lhsT=wt[:, i, P:DC],
                            rhs=xT[:, i, 512 * qi:512 * qi + 512],
                            start=(i == 0),
                            stop=(i == nsteps - 1),
                        )
                return f

            for i in range(nsteps):
                steps.append(mk_mm(i))

            def cp():
                nc.vector.tensor_copy(
                    dst[:, 1, 512 * qi:512 * qi + 512], pq_state["t"][:]
                )
            steps.append(cp)
            return steps

        mb1_steps = []
        for qi in range(4):
            mb1_steps.extend(make_proj_quarter(wq_t, qcT, qi, "q", fp8=True))
        mb1_steps.append(lambda: rope_q_half(1, 0))
        mb1_steps.append(lambda: rope_q_half(1, 1))
        for qi in range(4):
            mb1_steps.extend(make_proj_quarter(wv_t, vT, qi, "v"))
        for sb in range(NSB):
            mb1_steps.append(mk_tp(sb, 1))
        fillers.extend(mb1_steps)

        def v_mb0_section():
            # V mb0 + all 16 transposes, emitted AFTER the burst passes'
            # scores so the exp stream is never queued behind the
            # x^T-DMA-gated V matmuls in the PE FIFO.
            # sc_ps holds 4 banks during the burst passes, so va (4) and
            # tp (3) must be sequential, not nested
            with ExitStack() as vctx:
                v_ps = vctx.enter_context(
                    tc.tile_pool(name="v_ps", bufs=1, space="PSUM")
                )
                va = v_ps.tile([P, S], f32, tag="va")
                for i in range(NEC):
                    for qt in range(4):
                        nc.tensor.matmul(
                            va[:, qt * 512:(qt + 1) * 512],
                            lhsT=wv_t[:, i, 0:P],
                            rhs=xT[:, i, qt * 512:(qt + 1) * 512],
                            start=(i == 0),
                            stop=(i == NEC - 1),
                        )
                nc.vector.tensor_copy(vT[:, 0, 0:1024], va[:, 0:1024])
                nc.vector.tensor_copy(vT[:, 0, 1024:2048], va[:, 1024:2048])
            with ExitStack() as vctx:
                tp_ps = vctx.enter_context(
                    tc.tile_pool(name="tp_ps", bufs=3, space="PSUM")
                )
                for sb in range(NSB):
                    tp = tp_ps.tile([P, P], bf16, tag="tp", name=f"tp0_{sb}")
                    nc.tensor.transpose(
                        tp[:], vT[:, 0, sb * P:(sb + 1) * P], ident[:]
                    )
                    nc.vector.tensor_copy(
                        vn_dst(sb, 0), tp[:].rearrange("p (a b) -> p a b", a=2)
                    )

        yq_state = {}

        def mk_y_half(sb, eh):
            e0 = 512 * eh

            def q1():
                yq_state["t"] = ycfg["pool"].tile(
                    [P, 512], f32, tag="pq", name=f"yq_{sb}_{eh}"
                )
                nc.tensor.matmul(
                    yq_state["t"][:],
                    lhsT=onrm[:, 0, sb * P:(sb + 1) * P],
                    rhs=wo_t[:, 0, e0:e0 + 512],
                    start=True,
                    stop=False,
                )

            def q2():
                nc.tensor.matmul(
                    yq_state["t"][:],
                    lhsT=onrm[:, 1, sb * P:(sb + 1) * P],
                    rhs=wo_t[:, 1, e0:e0 + 512],
                    start=False,
                    stop=True,
                )
                ys = ys_pool.tile([P, 512], bf16, tag="ys",
                                  name=f"ys_{sb}_{eh}")
                if ycfg["tail"] and eh == 1:
                    # post-attention: ACT + the scalar DMA ring are free
                    nc.scalar.copy(ys[:], yq_state["t"][:])
                    nc.scalar.dma_start(
                        y[sb * P:(sb + 1) * P, e0:e0 + 512], ys[:]
                    )
                else:
                    nc.vector.tensor_copy(ys[:], yq_state["t"][:])
                    nc.sync.dma_start(
                        y[sb * P:(sb + 1) * P, e0:e0 + 512], ys[:]
                    )

            return [q1, q2]

        # --- attention passes: h-major, single head per pass ---
        # Epilogue staging: den+recip (DVE) fire at bank completion;
        # the gpsimd broadcast + DVE multiply are DEFERRED a couple of
        # iterations so they reach their FIFOs with inputs long ready
        # and never head-of-line-block the PV path.
        deferred = deque()

        def run_deferred():
            while deferred:
                deferred.popleft()()

        def attention_pass(h, pss, after_post0=None, defer_pv=False, nf=2):
            mb, par = h // 2, h % 2
            q0 = pss * 1024
            nti = 8 if pss == 0 else 16
            stopA = (q0 + 512) // P - 1
            stopB = (q0 + 1024) // P - 1
            acc_state = {}

            def get_accs():
                # acc banks are allocated lazily: for deferred-PV passes
                # the PSUM pools don't exist yet at scores/exp time
                if "a" not in acc_state:
                    acc_state["a"] = pools["accA"].tile(
                        [65, 512], f32, tag="accA", name=f"accA_{h}_{pss}"
                    )
                    acc_state["b"] = pools["accB"].tile(
                        [65, 512], f32, tag="accB", name=f"accB_{h}_{pss}"
                    )
                return acc_state["a"], acc_state["b"]

            def issue_pv(pt, w0, ti):
                accs = get_accs()
                for bk in range(2):
                    lo = max(w0, q0 + 512 * bk)
                    hi = q0 + 512 * (bk + 1)
                    if lo >= hi:
                        continue
                    b0 = q0 + 512 * bk
                    nc.tensor.matmul(
                        accs[bk][:, lo - b0:hi - b0],
                        lhsT=vn[:, ti, h, :],
                        rhs=pt[:, lo - q0:hi - q0],
                        start=(ti == 0),
                        stop=(ti == (stopA if bk == 0 else stopB)),
                    )

            def norm_pre(bk):
                acc = get_accs()[bk]
                den = dn.tile([1, 512], f32, tag="den",
                              name=f"den_{h}_{pss}_{bk}")
                nc.vector.tensor_copy(den[:], acc[64:65, :])
                rden = dn.tile([1, 512], f32, tag="rden",
                               name=f"rden_{h}_{pss}_{bk}")
                nc.vector.reciprocal_approx_fast(rden[:], den[:])
                return rden

            def norm_post(bk, rden):
                gcol = q0 + 512 * bk
                acc = get_accs()[bk]
                rdb = dn.tile([DH, 512], f32, tag="rdb",
                              name=f"rdb_{h}_{pss}_{bk}")
                nc.gpsimd.partition_broadcast(rdb[:], rden[:])
                nc.vector.tensor_mul(
                    onrm[par * DH:par * DH + DH, mb, gcol:gcol + 512],
                    acc[0:DH, :],
                    rdb[:],
                )

            def finish(pvs):
                for args in pvs:
                    issue_pv(*args)
                    if args[2] == stopA:
                        rden = norm_pre(0)

                        def post0(rden=rden):
                            norm_post(0, rden)
                            if after_post0 is not None:
                                after_post0()
                        deferred.append(post0)
                rden = norm_pre(1)
                deferred.append(lambda rden=rden: norm_post(1, rden))

            pending = None
            pvs = []
            for ti in range(nti):
                t0 = ti * P
                w0 = max(t0, q0)
                width = q0 + 1024 - w0
                diag = t0 >= q0
                sc = sc_ps.tile([P, 1024], f32, tag="sc",
                                name=f"sc_{h}_{pss}_{ti}")
                d0 = 1024 - width
                kblk = kT[:, mb, t0:t0 + P]
                if diag:
                    # additive causal mask: preload MASK_VAL above the
                    # diagonal into the diag 128-col piece (identity-
                    # transpose matmul), then accumulate scores onto it.
                    nc.tensor.matmul(
                        sc[:, d0:d0 + P], lhsT=ident[:], rhs=msk_t[:],
                        start=True, stop=False,
                    )
                    nc.tensor.matmul(
                        sc[:, d0:d0 + P],
                        lhsT=kblk,
                        rhs=qz[:, mb, par, q0 + d0:q0 + d0 + P],
                        start=False, stop=True,
                    )
                    p0 = d0 + P
                else:
                    p0 = d0
                while p0 < 1024:
                    p1 = min(1024, (p0 // 512 + 1) * 512)
                    nc.tensor.matmul(
                        sc[:, p0:p1],
                        lhsT=kblk,
                        rhs=qz[:, mb, par, q0 + p0:q0 + p1],
                    )
                    p0 = p1
                if not defer_pv:
                    emit_fillers(nf)
                    if deferred:
                        deferred.popleft()()
                pt = ptp.tile([P, 1024], bf16, tag="pt",
                              name=f"pt_{h}_{pss}_{ti}")
                nc.scalar.activation(
                    pt[:, d0:1024], sc[:, d0:1024], AF.Exp, scale=EXP_SCALE
                )
                if defer_pv:
                    pvs.append((pt, w0, ti))
                    continue
                if pending is not None:
                    issue_pv(*pending)
                    if pending[2] == stopA:
                        rden = norm_pre(0)

                        def post0(rden=rden):
                            norm_post(0, rden)
                            if after_post0 is not None:
                                after_post0()
                        deferred.append(post0)
                pending = (pt, w0, ti)
            if defer_pv:
                return lambda: finish(pvs)
            issue_pv(*pending)
            rden = norm_pre(1)
            deferred.append(lambda rden=rden: norm_post(1, rden))
            emit_fillers(2)
            return None

        def y_after_stopA():
            for sb in range(8, 12):
                for eh in range(2):
                    fillers.extend(mk_y_half(sb, eh))

        # --- pass sequence ---
        # h0/h1 pass0 run first with PV fully deferred: their exps need
        # only kT/qz mb0 (fp8 x path, ready ~18us), while V mb0 waits on
        # the 4MB bf16 x^T DMA (~35us). The V section + PV drains slot in
        # behind those 16 score matmuls on the PE FIFO; the ACT exp
        # stream never queues behind the DMA-gated V work.
        d00 = attention_pass(0, 0, defer_pv=True)
        d10 = attention_pass(1, 0, defer_pv=True)
        v_mb0_section()
        pools["accA"] = attctx.enter_context(
            tc.tile_pool(name="accA_ps", bufs=1, space="PSUM")
        )
        pools["accB"] = attctx.enter_context(
            tc.tile_pool(name="accB_ps", bufs=2, space="PSUM")
        )
        pools["yq"] = attctx.enter_context(
            tc.tile_pool(name="yq_ps", bufs=1, space="PSUM")
        )
        ycfg["pool"] = pools["yq"]
        d00()
        # h0p0's deferred norm_posts must be EMITTED before h1p0's PV
        # drain reuses the accA bank (Tile deps only point backward)
        run_deferred()
        d10()
        attention_pass(0, 1)
        attention_pass(1, 1)
        # everything heads 2/3 read (kT/qz/vn mb1) must be emitted
        # before their passes' reads
        flush_fillers()
        attention_pass(2, 0)
        attention_pass(2, 1)
        attention_pass(3, 0)
        run_deferred()
        # only sb0-3 drain in-pass: the single yq PSUM bank serializes a
        # y quantum at ~1.6us, so more would block h3p1's own scores
        for sb in range(4):
            for eh in range(2):
                fillers.extend(mk_y_half(sb, eh))
        attention_pass(3, 1)
        # the deferred final norm_post must be emitted while the acc
        # pools are still open (their banks are reused by the tail pool)
        run_deferred()
        # tail: attention PSUM pools close; the remaining W_o blocks
        # drain through an 8-deep PSUM pool so the matmul->copy->DMA
        # chains pipeline instead of serializing on one bank
        attctx.close()
        tail_ps = ctx.enter_context(
            tc.tile_pool(name="tail_ps", bufs=8, space="PSUM")
        )
        ycfg["pool"] = tail_ps
        ycfg["tail"] = True
        for sb in range(4, NSB):
            for eh in range(2):
                fillers.extend(mk_y_half(sb, eh))
        flush_fillers()

        if debug:
            nc.sync.dma_start(dbg["dkT"], kT[:])
            nc.sync.dma_start(dbg["dqz"], qz[:])
            nc.sync.dma_start(dbg["dvT"], vT[:])
            nc.sync.dma_start(dbg["dvn"], vn[:])
            nc.sync.dma_start(dbg["donrm"], onrm[:])

    nc.compile()
    return nc


def get_program():
    global _PROG
    if _PROG is None:
        _PROG = _build_program()
    return _PROG


def make_in_maps(x, W_q, W_k, W_v, W_o):
    perm = _perm64()
    idx_local = (np.arange(DC) // 64) * 64 + perm[np.arange(DC) % 64]
    ang, sgn = _cos_sin_tiles()
    cos_np = np.cos(ang).astype(BF16)
    sin_np = (sgn * np.sin(ang)).astype(BF16)
    # scores tile is (t, q): additive causal mask, 0 where t <= q
    # (keep), MASK_VAL where t > q (exp -> 0)
    cmask_np = np.where(np.triu(np.ones((P, P))) > 0, 0.0,
                        MASK_VAL).astype(BF16)

    def pair8(a):
        # [E, M] -> [128, 4, 2, M] fp8 DoubleRow pair layout
        e, m = a.shape
        return np.ascontiguousarray(
            np.clip(a, -448, 448).reshape(4, 2, P, m).transpose(2, 0, 1, 3)
        ).astype(F8)

    in_maps = []
    for c in range(NCORES):
        b, hg = c // 4, c % 4
        base = hg * DC
        xt = x[b].T
        in_maps.append(
            dict(
                xbt=np.ascontiguousarray(xt.astype(BF16)),
                xbt8=pair8(xt),
                wq8=pair8(W_q[:, base + idx_local] * QK_SCALE),
                wk8=pair8(W_k[:, base + idx_local] * QK_SCALE),
                wv=np.ascontiguousarray(W_v[:, base:base + DC].astype(BF16)),
                wo=np.ascontiguousarray(W_o[base:base + DC, :].astype(BF16)),
                cosr=cos_np,
                sinr=sin_np,
                cmask=cmask_np,
            )
        )
    return in_maps


def kernel(x, W_q, W_k, W_v, W_o, _trace=False, _trace_cores=None):
    from concourse.bass_utils import run_bass_kernel_spmd

    x = np.asarray(x, dtype=np.float32)
    W_q = np.asarray(W_q, dtype=np.float32)
    W_k = np.asarray(W_k, dtype=np.float32)
    W_v = np.asarray(W_v, dtype=np.float32)
    W_o = np.asarray(W_o, dtype=np.float32)

    nc = get_program()
    in_maps = make_in_maps(x, W_q, W_k, W_v, W_o)
    res = run_bass_kernel_spmd(
        nc,
        in_maps,
        list(range(NCORES)),
        trace=_trace,
        trace_cores=_trace_cores,
    )
    y = np.zeros((B, S, E), np.float32)
    for c in range(NCORES):
        y[c // 4] += np.asarray(res.results[c]["y"], dtype=np.float32)
    if _trace:
        return y, res
    return y


# revision 31
# speedup vs baseline: 1.0249x; 1.0249x over previous
"""Multi-head causal attention with RoPE on 8 Trainium2 NeuronCores.

Sharding: data-parallel over batch (B=2) x tensor-parallel over heads
(16 heads -> 4 groups of 4). Core c handles batch c//4, heads
[(c%4)*4, (c%4)*4+4). Each core computes a partial y = attn_out @ W_o
for its head group; the host sums the 4 partials per batch (the "W_o
all-reduce").

v4 design (v2 dtypes + early-start scheduling):
  - Q/K projections in fp8 DoubleRow from an fp8 x^T copy, with the
    fp8 weights pre-scaled x16 (escapes fp8e4m3 subnormals; the exp
    scale absorbs 1/256). The value path (V, probs, o, W_o) stays bf16:
    fp8 noise in a dot-product operand does NOT average down, and
    value-path fp8 measured ~6% output error.
  - Junk warm-up matmuls at t=0 hold the PE HAM clock gate open through
    the input-DMA window, and the prologue is restructured to unblock
    the first exp at ~14us (vs ~51us): K/Q mb0 chunk-major over the
    arriving fp8 x chunks -> rope half0 immediately; V mb0 chunk-major
    over the arriving bf16 x chunks; only transposes sb0-7 run before
    attention (sb8-15 + all of mb1 are fillers).
  - V-block transposes for the filler phase go through the DMA xbar
    (dma_start_transpose) instead of TensorE, freeing ~6us of PE time
    in the attention phase where PE is the co-bottleneck with ACT.
  - Attention: one head per pass, q in 1024-col passes, scores^T[t,q]
    as a single matmul per t-block, exp on ACT (the pass pacer), PV
    software-pipelined one iteration behind, denominator as a 65th
    ones-column of V. Causal mask preloaded additively (-1e7) via an
    identity-transpose matmul so the exp->PV path stays clean.
  - Epilogue per 512-col PSUM bank: den copy -> reciprocal_approx_fast
    -> gpsimd partition_broadcast -> one DVE multiply from PSUM.
  - y (o @ W_o) quanta interleave as fillers: sb0-7 during the last
    head's second pass, sb8-11 appended mid-pass once that pass's
    bank-A epilogue lands, so the tail only drains sb12-15.
"""

import os
import sys
from collections import deque
from contextlib import ExitStack

import numpy as np

for _p in ("/opt/trn_rl_repo",):
    if os.path.isdir(_p) and _p not in sys.path:
        sys.path.insert(0, _p)

import ml_dtypes  # noqa: E402

BF16 = ml_dtypes.bfloat16
F8 = ml_dtypes.float8_e4m3fn

B, S, E = 2, 2048, 1024
H, DH = 16, 64
NCORES = 8
HPC = H // 4          # 4 heads per core
DC = HPC * DH         # 256 head dims per core
ATTN_SCALE = 1.0 / 32.0  # 1/sqrt(E)
ROPE_BASE = 10000.0
P = 128
NSB = S // P          # 16 sequence blocks
NEC = E // P          # 8 E chunks
MB = DC // P          # 2 partition blocks of head dims

QK_SCALE = 16.0       # host pre-scale on W_q/W_k (fp8 subnormal escape)
EXP_SCALE = ATTN_SCALE / (QK_SCALE * QK_SCALE)
MASK_VAL = -1e7       # additive causal mask; * EXP_SCALE -> exp underflows to 0

DMA_TP = os.environ.get("KNL_DMATP", "1") == "1"

_PROG = None


def _perm64():
    """perm[j] = original head-dim index stored at permuted position j.

    Quadrant q of the permuted layout holds RoPE pairs i in
    [16q, 16q+16): even elements (2i) at slots 0-15, odd (2i+1) at
    slots 16-31. The rotation partner is then always +-16 partitions
    away within one 32-partition quadrant (stream_shuffle range).
    """
    j = np.arange(64)
    qd, r = j // 32, j % 32
    i = 16 * qd + (r % 16)
    return 2 * i + (r >= 16)


def _cos_sin_tiles():
    pl = np.arange(P) % 64
    qd, r = pl // 32, pl % 32
    i = 16 * qd + (r % 16)
    inv = ROPE_BASE ** (-(2.0 * i) / DH)
    ang = np.arange(S)[None, :] * inv[:, None]          # (128, S)
    sgn = np.where(r < 16, -1.0, 1.0)[:, None]
    return ang, sgn


def _build_program(debug=False):
    import concourse.bacc as bacc
    import concourse.tile as tile
    from concourse import masks, mybir

    f32 = mybir.dt.float32
    bf16 = mybir.dt.bfloat16
    f8 = mybir.dt.float8e4
    AF = mybir.ActivationFunctionType
    DR = mybir.MatmulPerfMode.DoubleRow

    nc = bacc.Bacc("TRN2", target_bir_lowering=False, debug=False)
    xbt = nc.dram_tensor("xbt", [E, S], bf16, kind="ExternalInput").ap()
    # fp8 copies of x^T / W_q / W_k in DoubleRow pair layout
    # [p, chunk-pair i, j, *]: element (2i+j)*128+p of the E axis
    xbt8 = nc.dram_tensor("xbt8", [P, NEC // 2, 2, S], f8,
                          kind="ExternalInput").ap()
    wq8 = nc.dram_tensor("wq8", [P, NEC // 2, 2, DC], f8,
                         kind="ExternalInput").ap()
    wk8 = nc.dram_tensor("wk8", [P, NEC // 2, 2, DC], f8,
                         kind="ExternalInput").ap()
    wv = nc.dram_tensor("wv", [E, DC], bf16, kind="ExternalInput").ap()
    wo = nc.dram_tensor("wo", [DC, E], bf16, kind="ExternalInput").ap()
    cosr = nc.dram_tensor("cosr", [P, S], bf16, kind="ExternalInput").ap()
    sinr = nc.dram_tensor("sinr", [P, S], bf16, kind="ExternalInput").ap()
    cmask = nc.dram_tensor("cmask", [P, P], bf16, kind="ExternalInput").ap()
    y = nc.dram_tensor("y", [S, E], bf16, kind="ExternalOutput").ap()
    # tiny output read from the warm-up accumulator so neuronxcc cannot
    # dead-code-eliminate the junk matmuls that keep the HAM clock warm
    warm = nc.dram_tensor("warm", [1, 4], f32, kind="ExternalOutput").ap()
    dbg = {}
    if debug:
        for nm, shp in (
            ("dkT", [P, MB, S]), ("dqz", [P, MB, 2, S]), ("dvT", [P, MB, S]),
            ("dvn", [P, NSB, HPC, 65]), ("donrm", [P, MB, S]),
        ):
            dbg[nm] = nc.dram_tensor(nm, shp, bf16, kind="ExternalOutput").ap()

    with ExitStack() as ctx:
        tc = ctx.enter_context(tile.TileContext(nc))
        consts = ctx.enter_context(tc.tile_pool(name="consts", bufs=1))
        persist = ctx.enter_context(tc.tile_pool(name="persist", bufs=1))

        ident = consts.tile([P, P], bf16, tag="ident")
        junk = consts.tile([P, 512], bf16, tag="junk")
        wk_t = consts.tile([P, NEC // 2, 2, DC], f8, tag="wk")
        wq_t = consts.tile([P, NEC // 2, 2, DC], f8, tag="wq")
        wv_t = consts.tile([P, NEC, DC], bf16, tag="wv")
        wo_t = consts.tile([P, MB, E], bf16, tag="wo")
        cos_t = consts.tile([P, S], bf16, tag="cos")
        sin_t = consts.tile([P, S], bf16, tag="sin")
        msk_t = consts.tile([P, P], bf16, tag="msk")

        xT = persist.tile([P, NEC, S], bf16, tag="xT")
        xT8 = persist.tile([P, NEC // 2, 2, S], f8, tag="xT8")
        kcT = persist.tile([P, MB, S], bf16, tag="kcT")
        qcT = persist.tile([P, MB, S], bf16, tag="qcT")
        vT = persist.tile([P, MB, S], bf16, tag="vT")
        # RoPE'd K^T, and Q^T zero-padded per head parity: slice
        # [:, mb, par, :] has head (2*mb+par)'s 64 rows live and the
        # other 64 rows zero, so scores use the FULL 128-row K^T block
        # as lhsT (the HAM clock gate never grants full clock to
        # partial-height matmul streams; zero rows contribute 0).
        kT = persist.tile([P, MB, S], bf16, tag="kT")
        qz = persist.tile([P, MB, 2, S], bf16, tag="qz")
        vn = persist.tile([P, NSB, HPC, 65], bf16, tag="vn")
        onrm = persist.tile([P, MB, S], bf16, tag="onrm")

        # ---- t=0 setup: memsets, masks, warm-up, DMA streams ----
        nc.vector.memset(junk[:], 1.0)
        nc.vector.memset(vn[:, :, :, 64:65], 1.0)
        nc.gpsimd.memset(qz[0:DH, :, 1, :], 0.0)
        nc.gpsimd.memset(qz[DH:P, :, 0, :], 0.0)
        masks.make_identity(nc, ident[:])

        wv_r = wv.rearrange("(c p) m -> p c m", p=P)
        # The rings sustain ~130 GB/s each, so ordering is everything:
        # x8 (gates the exp stream via K/Q+rope) goes first on both
        # rings, then the rope tables, then the 4MB bf16 x^T (gates only
        # the deferred-PV value path), then late weights.
        # sync ring: wk8, x8 evens, sin h0, wvA(mb0), xbt evens, sin h1, wvB
        # scalar ring: wq8, x8 odds, cos h0, cmask, xbt odds, cos h1, wo
        nc.sync.dma_start(wk_t[:], wk8)
        nc.scalar.dma_start(wq_t[:], wq8)
        for ec in range(NEC // 2):
            eng = nc.sync if ec % 2 == 0 else nc.scalar
            eng.dma_start(xT8[:, ec, :, :], xbt8[:, ec, :, :])
        nc.sync.dma_start(sin_t[:, 0:1024], sinr[:, 0:1024])
        nc.scalar.dma_start(cos_t[:, 0:1024], cosr[:, 0:1024])
        nc.scalar.dma_start(msk_t[:], cmask)
        nc.sync.dma_start(wv_t[:, :, 0:P], wv_r[:, :, 0:P])
        for ec in range(NEC):
            eng = nc.sync if ec % 2 == 0 else nc.scalar
            eng.dma_start(xT[:, ec, :], xbt[ec * P:(ec + 1) * P, :])
            if ec == 1:
                nc.sync.dma_start(sin_t[:, 1024:2048], sinr[:, 1024:2048])
                nc.scalar.dma_start(cos_t[:, 1024:2048], cosr[:, 1024:2048])
        nc.sync.dma_start(wv_t[:, :, P:DC], wv_r[:, :, P:DC])
        nc.scalar.dma_start(wo_t[:], wo.rearrange("(c p) n -> p c n", p=P))

        # exp spline table preload (one-time ~1.3us) off the critical path
        scr = consts.tile([P, 16], f32, tag="scr")
        nc.scalar.activation(scr[:], junk[:, 0:16], AF.Exp, scale=EXP_SCALE)
        # gpsimd ANT-lib preload: partition_broadcast's custom library
        # loads once here (~9us, hidden under input DMA)
        scr2 = consts.tile([16, 16], f32, tag="scr2")
        scr3 = consts.tile([1, 16], f32, tag="scr3")
        nc.vector.memset(scr3[:], 1.0)
        nc.gpsimd.partition_broadcast(scr2[:], scr3[:])

        # warm-up matmuls: the PE must stay busy from t=0 until the first
        # x8 chunk lands (~13us: ~6us queue bootstrap + transfer), else
        # the HAM MID window re-throttles the clock to 1.2 GHz for the
        # whole prologue. One ACCUMULATION CHAIN whose result is DMA'd
        # to a tiny output, so neuronxcc cannot dead-code-eliminate it
        # (independent dead-write matmuls get pruned).
        with ExitStack() as wctx:
            wu_ps = wctx.enter_context(
                tc.tile_pool(name="wu_ps", bufs=1, space="PSUM")
            )
            wu = wu_ps.tile([P, 512], f32, tag="wu")
            for r in range(24):
                nc.tensor.matmul(
                    wu[:], lhsT=junk[:, 0:P], rhs=junk[:],
                    start=(r == 0), stop=(r == 23),
                )
            ws = consts.tile([1, 4], f32, tag="ws")
            nc.vector.tensor_copy(ws[:], wu[0:1, 0:4])
            nc.sync.dma_start(warm, ws[:])

        # ---- K/Q mb0 chunk-major over arriving x8 chunks ----
        # ka and qa live in separate pools: qa's 4 banks are released
        # right after the mb0 casts (the burst-phase score buffers land
        # there), while ka's banks are reused for the K mb1 projection,
        # which fills the otherwise-idle PE window between the mb0
        # projections and the rope-gated first scores.
        kqA = ExitStack()
        kqA_ps = kqA.enter_context(tc.tile_pool(name="kqA_ps", bufs=1,
                                                space="PSUM"))
        with ExitStack() as kqB:
            kqB_ps = kqB.enter_context(
                tc.tile_pool(name="kqB_ps", bufs=1, space="PSUM")
            )
            ka = kqA_ps.tile([P, S], f32, tag="ka")
            qa = kqB_ps.tile([P, S], f32, tag="qa")
            for i in range(NEC // 2):
                for wt, acc in ((wk_t, ka), (wq_t, qa)):
                    for qt in range(4):
                        nc.tensor.matmul(
                            acc[:, qt * 512:(qt + 1) * 512],
                            lhsT=wt[:, i, :, 0:P],
                            rhs=xT8[:, i, :, qt * 512:(qt + 1) * 512],
                            perf_mode=DR,
                            start=(i == 0),
                            stop=(i == NEC // 2 - 1),
                        )
            # PSUM -> bf16 SBUF; DVE takes only kcT half0 (the rope
            # critical chain), ACT (idle during the prologue) the rest
            nc.scalar.copy(qcT[:, 0, 0:1024], qa[:, 0:1024])
            nc.vector.tensor_copy(kcT[:, 0, 0:1024], ka[:, 0:1024])
            nc.scalar.copy(kcT[:, 0, 1024:2048], ka[:, 1024:2048])
            nc.scalar.copy(qcT[:, 0, 1024:2048], qa[:, 1024:2048])

        shuf_mask = list(range(16, 32)) + list(range(16))
        sh_pool = ctx.enter_context(tc.tile_pool(name="sh", bufs=2))

        def rope_k_half(mb, hf):
            sl = slice(1024 * hf, 1024 * hf + 1024)
            sh = sh_pool.tile([P, 1024], bf16, tag="shk", name=f"shk{mb}_{hf}")
            nc.vector.stream_shuffle(sh[:], kcT[:, mb, sl], shuf_mask)
            nc.vector.tensor_mul(sh[:], sh[:], sin_t[:, sl])
            nc.vector.tensor_mul(kT[:, mb, sl], kcT[:, mb, sl], cos_t[:, sl])
            nc.vector.tensor_add(kT[:, mb, sl], kT[:, mb, sl], sh[:])

        def rope_q_half(mb, hf):
            sl = slice(1024 * hf, 1024 * hf + 1024)
            sh = sh_pool.tile([P, 1024], bf16, tag="shq", name=f"shq{mb}_{hf}")
            nc.vector.stream_shuffle(sh[:], qcT[:, mb, sl], shuf_mask)
            nc.vector.tensor_mul(sh[:], sh[:], sin_t[:, sl])
            qr = sh_pool.tile([P, 1024], bf16, tag="qr", name=f"qr{mb}_{hf}")
            nc.vector.tensor_mul(qr[:], qcT[:, mb, sl], cos_t[:, sl])
            nc.vector.tensor_add(qr[:], qr[:], sh[:])
            nc.vector.tensor_copy(qz[0:DH, mb, 0, sl], qr[0:DH, :])
            nc.vector.tensor_copy(qz[DH:P, mb, 1, sl], qr[DH:P, :])

        # unblock h0 pass0 ASAP: only mb0 half0 rope is on its path
        rope_k_half(0, 0)
        rope_q_half(0, 0)

        # ---- K mb1 in the pre-burst PE-idle window (xT8 is resident) ----
        ka1 = kqA_ps.tile([P, S], f32, tag="ka", name="ka1")
        for i in range(NEC // 2):
            for qt in range(4):
                nc.tensor.matmul(
                    ka1[:, qt * 512:(qt + 1) * 512],
                    lhsT=wk_t[:, i, :, P:DC],
                    rhs=xT8[:, i, :, qt * 512:(qt + 1) * 512],
                    perf_mode=DR,
                    start=(i == 0),
                    stop=(i == NEC // 2 - 1),
                )
        # casts on ACT (idle until the first exp); the DVE queue must stay
        # clear for the rope -> V-cast -> vn chain that gates PV
        nc.scalar.copy(kcT[:, 1, 0:1024], ka1[:, 0:1024])
        nc.scalar.copy(kcT[:, 1, 1024:2048], ka1[:, 1024:2048])
        kqA.close()

        def vn_dst(sb, mb):
            return vn[:, sb, 2 * mb:2 * mb + 2, 0:64]

        # rope halves1 go on the DVE queue now, ahead of the V-phase DVE
        # work (qz half1 is needed when h0 pass1 starts)
        rope_q_half(0, 1)
        rope_k_half(0, 1)

        # ---- attention + filler machinery ----
        # Fillers are work quanta interleaved into the ACT-paced
        # attention iterations. CRITICAL: a filler that WRITES data read
        # by a later pass must be emitted (program order) before that
        # pass's reads -- Tile deps only point backward -- so the deque
        # has a hard flush point before h==2.
        fillers = deque()

        def emit_fillers(n):
            for _ in range(n):
                if not fillers:
                    return
                fillers.popleft()()

        def flush_fillers():
            while fillers:
                fillers.popleft()()

        attctx = ExitStack()
        sc_ps = attctx.enter_context(
            tc.tile_pool(name="sc_ps", bufs=2, space="PSUM")
        )
        # deep probs buffering: the first two passes run with PV fully
        # deferred (their V^T blocks arrive only at ~x^T-DMA completion),
        # so up to 16 pt tiles are alive at once
        ptp = ctx.enter_context(tc.tile_pool(name="ptp", bufs=18))
        dn = ctx.enter_context(tc.tile_pool(name="dn", bufs=2))
        pools = {}
        ycfg = {"pool": None, "tail": False}
        ys_pool = ctx.enter_context(tc.tile_pool(name="ys", bufs=4))
        vs_pool = ctx.enter_context(tc.tile_pool(name="vs", bufs=3))

        # --- filler generators (transposes, proj quarters, y halves) ---
        def mk_tp(sb, mb):
            if DMA_TP:
                def f():
                    # DMA-xbar transpose into contiguous staging, then a
                    # DVE copy into vn's 65-wide slot layout; zero PE.
                    eng = nc.sync if sb % 2 == 0 else nc.scalar
                    vs = vs_pool.tile([P, P], bf16, tag="vs",
                                      name=f"vs{mb}_{sb}")
                    eng.dma_start_transpose(
                        vs[:], vT[:, mb, sb * P:(sb + 1) * P]
                    )
                    nc.vector.tensor_copy(
                        vn_dst(sb, mb),
                        vs[:].rearrange("p (a b) -> p a b", a=2),
                    )
            else:
                def f():
                    tp = pools["yq"].tile([P, P], bf16, tag="pq",
                                          name=f"tp{mb}s_{sb}")
                    nc.tensor.transpose(
                        tp[:], vT[:, mb, sb * P:(sb + 1) * P], ident[:]
                    )
                    nc.vector.tensor_copy(
                        vn_dst(sb, mb), tp[:].rearrange("p (a b) -> p a b", a=2)
                    )
            return f

        pq_state = {}

        def make_proj_quarter(wt, dst, qi, tagname, fp8=False):
            steps = []
            nsteps = NEC // 2 if fp8 else NEC

            def mk_mm(i):
                def f():
                    if i == 0:
                        pq_state["t"] = pools["yq"].tile(
                            [P, 512], f32, tag="pq",
                            name=f"pq_{tagname}_{qi}",
                        )
                    if fp8:
                        nc.tensor.matmul(
                            pq_state["t"][:],
                            lhsT=wt[:, i, :, P:DC],
                            rhs=xT8[:, i, :, 512 * qi:512 * qi + 512],
                            perf_mode=DR,
                            start=(i == 0),
                            stop=(i == nsteps - 1),
                        )
                    else:
                        nc.tensor.matmul(
                            pq_state["t"][:],
                            lhsT=wt[:, i, P:DC],
                            rhs=xT[:, i, 512 * qi:512 * qi + 512],
                            start=(i == 0),
                            stop=(i == nsteps - 1),
                        )
                return f

            for i in range(nsteps):
                steps.append(mk_mm(i))

            def cp():
                nc.vector.tensor_copy(
                    dst[:, 1, 512 * qi:512 * qi + 512], pq_state["t"][:]
                )
            steps.append(cp)
            return steps

        mb1_steps = []
        for qi in range(4):
            mb1_steps.extend(make_proj_quarter(wq_t, qcT, qi, "q", fp8=True))
        mb1_steps.append(lambda: rope_q_half(1, 0))
        mb1_steps.append(lambda: rope_q_half(1, 1))
        for qi in range(4):
            mb1_steps.extend(make_proj_quarter(wv_t, vT, qi, "v"))
        for sb in range(NSB):
            mb1_steps.append(mk_tp(sb, 1))
        fillers.extend(mb1_steps)

        def v_mb0_section():
            # V mb0 + all 16 transposes, emitted AFTER the burst passes'
            # scores so the exp stream is never queued behind the
            # x^T-DMA-gated V matmuls in the PE FIFO.
            # sc_ps holds 4 banks during the burst passes, so va (4) and
            # tp (3) must be sequential, not nested
            with ExitStack() as vctx:
                v_ps = vctx.enter_context(
                    tc.tile_pool(name="v_ps", bufs=1, space="PSUM")
                )
                va = v_ps.tile([P, S], f32, tag="va")
                for i in range(NEC):
                    for qt in range(4):
                        nc.tensor.matmul(
                            va[:, qt * 512:(qt + 1) * 512],
                            lhsT=wv_t[:, i, 0:P],
                            rhs=xT[:, i, qt * 512:(qt + 1) * 512],
                            start=(i == 0),
                            stop=(i == NEC - 1),
                        )
                nc.vector.tensor_copy(vT[:, 0, 0:1024], va[:, 0:1024])
                nc.vector.tensor_copy(vT[:, 0, 1024:2048], va[:, 1024:2048])
            with ExitStack() as vctx:
                tp_ps = vctx.enter_context(
                    tc.tile_pool(name="tp_ps", bufs=3, space="PSUM")
                )
                for sb in range(NSB):
                    tp = tp_ps.tile([P, P], bf16, tag="tp", name=f"tp0_{sb}")
                    nc.tensor.transpose(
                        tp[:], vT[:, 0, sb * P:(sb + 1) * P], ident[:]
                    )
                    nc.vector.tensor_copy(
                        vn_dst(sb, 0), tp[:].rearrange("p (a b) -> p a b", a=2)
                    )

        yq_state = {}

        def mk_y_half(sb, eh):
            e0 = 512 * eh

            def q1():
                yq_state["t"] = ycfg["pool"].tile(
                    [P, 512], f32, tag="pq", name=f"yq_{sb}_{eh}"
                )
                nc.tensor.matmul(
                    yq_state["t"][:],
                    lhsT=onrm[:, 0, sb * P:(sb + 1) * P],
                    rhs=wo_t[:, 0, e0:e0 + 512],
                    start=True,
                    stop=False,
                )

            def q2():
                nc.tensor.matmul(
                    yq_state["t"][:],
                    lhsT=onrm[:, 1, sb * P:(sb + 1) * P],
                    rhs=wo_t[:, 1, e0:e0 + 512],
                    start=False,
                    stop=True,
                )
                ys = ys_pool.tile([P, 512], bf16, tag="ys",
                                  name=f"ys_{sb}_{eh}")
                if ycfg["tail"] and eh == 1:
                    # post-attention: ACT + the scalar DMA ring are free
                    nc.scalar.copy(ys[:], yq_state["t"][:])
                    nc.scalar.dma_start(
                        y[sb * P:(sb + 1) * P, e0:e0 + 512], ys[:]
                    )
                else:
                    nc.vector.tensor_copy(ys[:], yq_state["t"][:])
                    nc.sync.dma_start(
                        y[sb * P:(sb + 1) * P, e0:e0 + 512], ys[:]
                    )

            return [q1, q2]

        # --- attention passes: h-major, single head per pass ---
        # Epilogue staging: den+recip (DVE) fire at bank completion;
        # the gpsimd broadcast + DVE multiply are DEFERRED a couple of
        # iterations so they reach their FIFOs with inputs long ready
        # and never head-of-line-block the PV path.
        deferred = deque()

        def run_deferred():
            while deferred:
                deferred.popleft()()

        def attention_pass(h, pss, after_post0=None, defer_pv=False, nf=2):
            mb, par = h // 2, h % 2
            q0 = pss * 1024
            nti = 8 if pss == 0 else 16
            stopA = (q0 + 512) // P - 1
            stopB = (q0 + 1024) // P - 1
            acc_state = {}

            def get_accs():
                # acc banks are allocated lazily: for deferred-PV passes
                # the PSUM pools don't exist yet at scores/exp time
                if "a" not in acc_state:
                    acc_state["a"] = pools["accA"].tile(
                        [65, 512], f32, tag="accA", name=f"accA_{h}_{pss}"
                    )
                    acc_state["b"] = pools["accB"].tile(
                        [65, 512], f32, tag="accB", name=f"accB_{h}_{pss}"
                    )
                return acc_state["a"], acc_state["b"]

            def issue_pv(pt, w0, ti):
                accs = get_accs()
                for bk in range(2):
                    lo = max(w0, q0 + 512 * bk)
                    hi = q0 + 512 * (bk + 1)
                    if lo >= hi:
                        continue
                    b0 = q0 + 512 * bk
                    nc.tensor.matmul(
                        accs[bk][:, lo - b0:hi - b0],
                        lhsT=vn[:, ti, h, :],
                        rhs=pt[:, lo - q0:hi - q0],
                        start=(ti == 0),
                        stop=(ti == (stopA if bk == 0 else stopB)),
                    )

            def norm_pre(bk):
                acc = get_accs()[bk]
                den = dn.tile([1, 512], f32, tag="den",
                              name=f"den_{h}_{pss}_{bk}")
                nc.vector.tensor_copy(den[:], acc[64:65, :])
                rden = dn.tile([1, 512], f32, tag="rden",
                               name=f"rden_{h}_{pss}_{bk}")
                nc.vector.reciprocal_approx_fast(rden[:], den[:])
                return rden

            def norm_post(bk, rden):
                gcol = q0 + 512 * bk
                acc = get_accs()[bk]
                rdb = dn.tile([DH, 512], f32, tag="rdb",
                              name=f"rdb_{h}_{pss}_{bk}")
                nc.gpsimd.partition_broadcast(rdb[:], rden[:])
                nc.vector.tensor_mul(
                    onrm[par * DH:par * DH + DH, mb, gcol:gcol + 512],
                    acc[0:DH, :],
                    rdb[:],
                )

            def finish(pvs):
                for args in pvs:
                    issue_pv(*args)
                    if args[2] == stopA:
                        rden = norm_pre(0)

                        def post0(rden=rden):
                            norm_post(0, rden)
                            if after_post0 is not None:
                                after_post0()
                        deferred.append(post0)
                rden = norm_pre(1)
                deferred.append(lambda rden=rden: norm_post(1, rden))

            pending = None
            pvs = []
            for ti in range(nti):
                t0 = ti * P
                w0 = max(t0, q0)
                width = q0 + 1024 - w0
                diag = t0 >= q0
                sc = sc_ps.tile([P, 1024], f32, tag="sc",
                                name=f"sc_{h}_{pss}_{ti}")
                d0 = 1024 - width
                kblk = kT[:, mb, t0:t0 + P]
                if diag:
                    # additive causal mask: preload MASK_VAL above the
                    # diagonal into the diag 128-col piece (identity-
                    # transpose matmul), then accumulate scores onto it.
                    nc.tensor.matmul(
                        sc[:, d0:d0 + P], lhsT=ident[:], rhs=msk_t[:],
                        start=True, stop=False,
                    )
                    nc.tensor.matmul(
                        sc[:, d0:d0 + P],
                        lhsT=kblk,
                        rhs=qz[:, mb, par, q0 + d0:q0 + d0 + P],
                        start=False, stop=True,
                    )
                    p0 = d0 + P
                else:
                    p0 = d0
                while p0 < 1024:
                    p1 = min(1024, (p0 // 512 + 1) * 512)
                    nc.tensor.matmul(
                        sc[:, p0:p1],
                        lhsT=kblk,
                        rhs=qz[:, mb, par, q0 + p0:q0 + p1],
                    )
                    p0 = p1
                if not defer_pv:
                    emit_fillers(nf)
                    if deferred:
                        deferred.popleft()()
                pt = ptp.tile([P, 1024], bf16, tag="pt",
                              name=f"pt_{h}_{pss}_{ti}")
                nc.scalar.activation(
                    pt[:, d0:1024], sc[:, d0:1024], AF.Exp, scale=EXP_SCALE
                )
                if defer_pv:
                    pvs.append((pt, w0, ti))
                    continue
                if pending is not None:
                    issue_pv(*pending)
                    if pending[2] == stopA:
                        rden = norm_pre(0)

                        def post0(rden=rden):
                            norm_post(0, rden)
                            if after_post0 is not None:
                                after_post0()
                        deferred.append(post0)
                pending = (pt, w0, ti)
            if defer_pv:
                return lambda: finish(pvs)
            issue_pv(*pending)
            rden = norm_pre(1)
            deferred.append(lambda rden=rden: norm_post(1, rden))
            emit_fillers(2)
            return None

        def y_after_stopA():
            for sb in range(8, 12):
                for eh in range(2):
                    fillers.extend(mk_y_half(sb, eh))

        # --- pass sequence ---
        # h0/h1 pass0 run first with PV fully deferred: their exps need
        # only kT/qz mb0 (fp8 x path, ready ~18us), while V mb0 waits on
        # the 4MB bf16 x^T DMA (~35us). The V section + PV drains slot in
        # behind those 16 score matmuls on the PE FIFO; the ACT exp
        # stream never queues behind the DMA-gated V work.
        d00 = attention_pass(0, 0, defer_pv=True)
        d10 = attention_pass(1, 0, defer_pv=True)
        v_mb0_section()
        # K mb1 rope on the DVE queue behind the vn copies (needed only
        # from head 2)
        rope_k_half(1, 0)
        rope_k_half(1, 1)
        pools["accA"] = attctx.enter_context(
            tc.tile_pool(name="accA_ps", bufs=1, space="PSUM")
        )
        pools["accB"] = attctx.enter_context(
            tc.tile_pool(name="accB_ps", bufs=2, space="PSUM")
        )
        pools["yq"] = attctx.enter_context(
            tc.tile_pool(name="yq_ps", bufs=1, space="PSUM")
        )
        ycfg["pool"] = pools["yq"]
        d00()
        # h0p0's deferred norm_posts must be EMITTED before h1p0's PV
        # drain reuses the accA bank (Tile deps only point backward)
        run_deferred()
        d10()
        attention_pass(0, 1)
        attention_pass(1, 1)
        # everything heads 2/3 read (kT/qz/vn mb1) must be emitted
        # before their passes' reads
        flush_fillers()
        attention_pass(2, 0)
        attention_pass(2, 1)
        attention_pass(3, 0)
        run_deferred()
        # only sb0-3 drain in-pass: the single yq PSUM bank serializes a
        # y quantum at ~1.6us, so more would block h3p1's own scores
        for sb in range(4):
            for eh in range(2):
                fillers.extend(mk_y_half(sb, eh))
        attention_pass(3, 1)
        # the deferred final norm_post must be emitted while the acc
        # pools are still open (their banks are reused by the tail pool)
        run_deferred()
        # tail: attention PSUM pools close; the remaining W_o blocks
        # drain through an 8-deep PSUM pool so the matmul->copy->DMA
        # chains pipeline instead of serializing on one bank
        attctx.close()
        tail_ps = ctx.enter_context(
            tc.tile_pool(name="tail_ps", bufs=8, space="PSUM")
        )
        ycfg["pool"] = tail_ps
        ycfg["tail"] = True
        for sb in range(4, NSB):
            for eh in range(2):
                fillers.extend(mk_y_half(sb, eh))
        flush_fillers()

        if debug:
            nc.sync.dma_start(dbg["dkT"], kT[:])
            nc.sync.dma_start(dbg["dqz"], qz[:])
            nc.sync.dma_start(dbg["dvT"], vT[:])
            nc.sync.dma_start(dbg["dvn"], vn[:])
            nc.sync.dma_start(dbg["donrm"], onrm[:])

    nc.compile()
    return nc


def get_program():
    global _PROG
    if _PROG is None:
        _PROG = _build_program()
    return _PROG


def make_in_maps(x, W_q, W_k, W_v, W_o):
    perm = _perm64()
    idx_local = (np.arange(DC) // 64) * 64 + perm[np.arange(DC) % 64]
    ang, sgn = _cos_sin_tiles()
    cos_np = np.cos(ang).astype(BF16)
    sin_np = (sgn * np.sin(ang)).astype(BF16)
    # scores tile is (t, q): additive causal mask, 0 where t <= q
    # (keep), MASK_VAL where t > q (exp -> 0)
    cmask_np = np.where(np.triu(np.ones((P, P))) > 0, 0.0,
                        MASK_VAL).astype(BF16)

    def pair8(a):
        # [E, M] -> [128, 4, 2, M] fp8 DoubleRow pair layout
        e, m = a.shape
        return np.ascontiguousarray(
            np.clip(a, -448, 448).reshape(4, 2, P, m).transpose(2, 0, 1, 3)
        ).astype(F8)

    in_maps = []
    for c in range(NCORES):
        b, hg = c // 4, c % 4
        base = hg * DC
        xt = x[b].T
        in_maps.append(
            dict(
                xbt=np.ascontiguousarray(xt.astype(BF16)),
                xbt8=pair8(xt),
                wq8=pair8(W_q[:, base + idx_local] * QK_SCALE),
                wk8=pair8(W_k[:, base + idx_local] * QK_SCALE),
                wv=np.ascontiguousarray(W_v[:, base:base + DC].astype(BF16)),
                wo=np.ascontiguousarray(W_o[base:base + DC, :].astype(BF16)),
                cosr=cos_np,
                sinr=sin_np,
                cmask=cmask_np,
            )
        )
    return in_maps


def kernel(x, W_q, W_k, W_v, W_o, _trace=False, _trace_cores=None):
    from concourse.bass_utils import run_bass_kernel_spmd

    x = np.asarray(x, dtype=np.float32)
    W_q = np.asarray(W_q, dtype=np.float32)
    W_k = np.asarray(W_k, dtype=np.float32)
    W_v = np.asarray(W_v, dtype=np.float32)
    W_o = np.asarray(W_o, dtype=np.float32)

    nc = get_program()
    in_maps = make_in_maps(x, W_q, W_k, W_v, W_o)
    res = run_bass_kernel_spmd(
        nc,
        in_maps,
        list(range(NCORES)),
        trace=_trace,
        trace_cores=_trace_cores,
    )
    y = np.zeros((B, S, E), np.float32)
    for c in range(NCORES):
        y[c // 4] += np.asarray(res.results[c]["y"], dtype=np.float32)
    if _trace:
        return y, res
    return y


# revision 36
# speedup vs baseline: 1.1750x; 1.1465x over previous
"""Multi-head causal attention with RoPE on 8 Trainium2 NeuronCores.

Sharding: data-parallel over batch (B=2) x tensor-parallel over heads
(16 heads -> 4 groups of 4). Core c handles batch c//4, heads
[(c%4)*4, (c%4)*4+4). Each core computes a partial y = attn_out @ W_o
for its head group; the host sums the 4 partials per batch (the "W_o
all-reduce").

v4 design (v2 dtypes + early-start scheduling):
  - Q/K projections in fp8 DoubleRow from an fp8 x^T copy, with the
    fp8 weights pre-scaled x16 (escapes fp8e4m3 subnormals; the exp
    scale absorbs 1/256). The value path (V, probs, o, W_o) stays bf16:
    fp8 noise in a dot-product operand does NOT average down, and
    value-path fp8 measured ~6% output error.
  - Junk warm-up matmuls at t=0 hold the PE HAM clock gate open through
    the input-DMA window, and the prologue is restructured to unblock
    the first exp at ~14us (vs ~51us): K/Q mb0 chunk-major over the
    arriving fp8 x chunks -> rope half0 immediately; V mb0 chunk-major
    over the arriving bf16 x chunks; only transposes sb0-7 run before
    attention (sb8-15 + all of mb1 are fillers).
  - V-block transposes for the filler phase go through the DMA xbar
    (dma_start_transpose) instead of TensorE, freeing ~6us of PE time
    in the attention phase where PE is the co-bottleneck with ACT.
  - Attention: one head per pass, q in 1024-col passes, scores^T[t,q]
    as a single matmul per t-block, exp on ACT (the pass pacer), PV
    software-pipelined one iteration behind, denominator as a 65th
    ones-column of V. Causal mask preloaded additively (-1e7) via an
    identity-transpose matmul so the exp->PV path stays clean.
  - Epilogue per 512-col PSUM bank: den copy -> reciprocal_approx_fast
    -> gpsimd partition_broadcast -> one DVE multiply from PSUM.
  - y (o @ W_o) quanta interleave as fillers: sb0-7 during the last
    head's second pass, sb8-11 appended mid-pass once that pass's
    bank-A epilogue lands, so the tail only drains sb12-15.
"""

import os
import sys
from collections import deque
from contextlib import ExitStack

import numpy as np

for _p in ("/opt/trn_rl_repo",):
    if os.path.isdir(_p) and _p not in sys.path:
        sys.path.insert(0, _p)

import ml_dtypes  # noqa: E402

BF16 = ml_dtypes.bfloat16
F8 = ml_dtypes.float8_e4m3fn

B, S, E = 2, 2048, 1024
H, DH = 16, 64
NCORES = 8
HPC = H // 4          # 4 heads per core
DC = HPC * DH         # 256 head dims per core
ATTN_SCALE = 1.0 / 32.0  # 1/sqrt(E)
ROPE_BASE = 10000.0
P = 128
NSB = S // P          # 16 sequence blocks
NEC = E // P          # 8 E chunks
MB = DC // P          # 2 partition blocks of head dims

QK_SCALE = 16.0       # host pre-scale on W_q/W_k (fp8 subnormal escape)
EXP_SCALE = ATTN_SCALE / (QK_SCALE * QK_SCALE)
MASK_VAL = -1e7       # additive causal mask; * EXP_SCALE -> exp underflows to 0

DMA_TP = os.environ.get("KNL_DMATP", "1") == "1"

_PROG = None


def _perm64():
    """perm[j] = original head-dim index stored at permuted position j.

    Quadrant q of the permuted layout holds RoPE pairs i in
    [16q, 16q+16): even elements (2i) at slots 0-15, odd (2i+1) at
    slots 16-31. The rotation partner is then always +-16 partitions
    away within one 32-partition quadrant (stream_shuffle range).
    """
    j = np.arange(64)
    qd, r = j // 32, j % 32
    i = 16 * qd + (r % 16)
    return 2 * i + (r >= 16)


def _cos_sin_tiles():
    pl = np.arange(P) % 64
    qd, r = pl // 32, pl % 32
    i = 16 * qd + (r % 16)
    inv = ROPE_BASE ** (-(2.0 * i) / DH)
    ang = np.arange(S)[None, :] * inv[:, None]          # (128, S)
    sgn = np.where(r < 16, -1.0, 1.0)[:, None]
    return ang, sgn


def _build_program(debug=False):
    import concourse.bacc as bacc
    import concourse.tile as tile
    from concourse import masks, mybir

    f32 = mybir.dt.float32
    bf16 = mybir.dt.bfloat16
    f8 = mybir.dt.float8e4
    AF = mybir.ActivationFunctionType
    DR = mybir.MatmulPerfMode.DoubleRow

    nc = bacc.Bacc("TRN2", target_bir_lowering=False, debug=False)
    xbt = nc.dram_tensor("xbt", [E, S], bf16, kind="ExternalInput").ap()
    # fp8 copies of x^T / W_q / W_k in DoubleRow pair layout
    # [p, chunk-pair i, j, *]: element (2i+j)*128+p of the E axis
    xbt8 = nc.dram_tensor("xbt8", [P, NEC // 2, 2, S], f8,
                          kind="ExternalInput").ap()
    wq8 = nc.dram_tensor("wq8", [P, NEC // 2, 2, DC], f8,
                         kind="ExternalInput").ap()
    wk8 = nc.dram_tensor("wk8", [P, NEC // 2, 2, DC], f8,
                         kind="ExternalInput").ap()
    wv = nc.dram_tensor("wv", [E, DC], bf16, kind="ExternalInput").ap()
    wo = nc.dram_tensor("wo", [DC, E], bf16, kind="ExternalInput").ap()
    cosr = nc.dram_tensor("cosr", [P, S], bf16, kind="ExternalInput").ap()
    sinr = nc.dram_tensor("sinr", [P, S], bf16, kind="ExternalInput").ap()
    cmask = nc.dram_tensor("cmask", [P, P], bf16, kind="ExternalInput").ap()
    y = nc.dram_tensor("y", [S, E], bf16, kind="ExternalOutput").ap()
    # tiny output read from the warm-up accumulator so neuronxcc cannot
    # dead-code-eliminate the junk matmuls that keep the HAM clock warm
    warm = nc.dram_tensor("warm", [1, 4], f32, kind="ExternalOutput").ap()
    dbg = {}
    if debug:
        for nm, shp in (
            ("dkT", [P, MB, S]), ("dqz", [P, MB, 2, S]), ("dvT", [P, MB, S]),
            ("dvn", [P, NSB, HPC, 65]), ("donrm", [P, MB, S]),
        ):
            dbg[nm] = nc.dram_tensor(nm, shp, bf16, kind="ExternalOutput").ap()

    with ExitStack() as ctx:
        tc = ctx.enter_context(tile.TileContext(nc))
        consts = ctx.enter_context(tc.tile_pool(name="consts", bufs=1))
        persist = ctx.enter_context(tc.tile_pool(name="persist", bufs=1))

        ident = consts.tile([P, P], bf16, tag="ident")
        junk = consts.tile([P, 512], bf16, tag="junk")
        wk_t = consts.tile([P, NEC // 2, 2, DC], f8, tag="wk")
        wq_t = consts.tile([P, NEC // 2, 2, DC], f8, tag="wq")
        wv_t = consts.tile([P, NEC, DC], bf16, tag="wv")
        wo_t = consts.tile([P, MB, E], bf16, tag="wo")
        cos_t = consts.tile([P, S], bf16, tag="cos")
        sin_t = consts.tile([P, S], bf16, tag="sin")
        msk_t = consts.tile([P, P], bf16, tag="msk")

        xT = persist.tile([P, NEC, S], bf16, tag="xT")
        xT8 = persist.tile([P, NEC // 2, 2, S], f8, tag="xT8")
        kcT = persist.tile([P, MB, S], bf16, tag="kcT")
        qcT = persist.tile([P, MB, S], bf16, tag="qcT")
        vT = persist.tile([P, MB, S], bf16, tag="vT")
        # RoPE'd K^T, and Q^T zero-padded per head parity: slice
        # [:, mb, par, :] has head (2*mb+par)'s 64 rows live and the
        # other 64 rows zero, so scores use the FULL 128-row K^T block
        # as lhsT (the HAM clock gate never grants full clock to
        # partial-height matmul streams; zero rows contribute 0).
        kT = persist.tile([P, MB, S], bf16, tag="kT")
        qz = persist.tile([P, MB, 2, S], bf16, tag="qz")
        vn = persist.tile([P, NSB, HPC, 65], bf16, tag="vn")
        onrm = persist.tile([P, MB, S], bf16, tag="onrm")

        # ---- t=0 setup: memsets, masks, warm-up, DMA streams ----
        nc.vector.memset(junk[:], 1.0)
        nc.vector.memset(vn[:, :, :, 64:65], 1.0)
        nc.gpsimd.memset(qz[0:DH, :, 1, :], 0.0)
        nc.gpsimd.memset(qz[DH:P, :, 0, :], 0.0)
        masks.make_identity(nc, ident[:])

        wv_r = wv.rearrange("(c p) m -> p c m", p=P)
        # The rings sustain ~130 GB/s each, so ordering is everything:
        # x8 (gates the exp stream via K/Q+rope) goes first on both
        # rings, then the rope tables, then the 4MB bf16 x^T (gates only
        # the deferred-PV value path), then late weights.
        # sync ring: wk8, x8 evens, sin h0, wvA(mb0), xbt evens, sin h1, wvB
        # scalar ring: wq8, x8 odds, cos h0, cmask, xbt odds, cos h1, wo
        nc.sync.dma_start(wk_t[:], wk8)
        nc.scalar.dma_start(wq_t[:], wq8)
        for ec in range(NEC // 2):
            eng = nc.sync if ec % 2 == 0 else nc.scalar
            eng.dma_start(xT8[:, ec, :, :], xbt8[:, ec, :, :])
        nc.sync.dma_start(sin_t[:, 0:1024], sinr[:, 0:1024])
        nc.scalar.dma_start(cos_t[:, 0:1024], cosr[:, 0:1024])
        nc.scalar.dma_start(msk_t[:], cmask)
        nc.sync.dma_start(wv_t[:, :, 0:P], wv_r[:, :, 0:P])
        for ec in range(NEC):
            eng = nc.sync if ec % 2 == 0 else nc.scalar
            eng.dma_start(xT[:, ec, :], xbt[ec * P:(ec + 1) * P, :])
            if ec == 1:
                nc.sync.dma_start(sin_t[:, 1024:2048], sinr[:, 1024:2048])
                nc.scalar.dma_start(cos_t[:, 1024:2048], cosr[:, 1024:2048])
        nc.sync.dma_start(wv_t[:, :, P:DC], wv_r[:, :, P:DC])
        nc.scalar.dma_start(wo_t[:], wo.rearrange("(c p) n -> p c n", p=P))

        # exp spline table preload (one-time ~1.3us) off the critical path
        scr = consts.tile([P, 16], f32, tag="scr")
        nc.scalar.activation(scr[:], junk[:, 0:16], AF.Exp, scale=EXP_SCALE)
        # gpsimd ANT-lib preload: partition_broadcast's custom library
        # loads once here (~9us, hidden under input DMA)
        scr2 = consts.tile([16, 16], f32, tag="scr2")
        scr3 = consts.tile([1, 16], f32, tag="scr3")
        nc.vector.memset(scr3[:], 1.0)
        nc.gpsimd.partition_broadcast(scr2[:], scr3[:])

        # warm-up matmuls: the PE must stay busy from t=0 until the first
        # x8 chunk lands (~13us: ~6us queue bootstrap + transfer), else
        # the HAM MID window re-throttles the clock to 1.2 GHz for the
        # whole prologue. One ACCUMULATION CHAIN whose result is DMA'd
        # to a tiny output, so neuronxcc cannot dead-code-eliminate it
        # (independent dead-write matmuls get pruned).
        with ExitStack() as wctx:
            wu_ps = wctx.enter_context(
                tc.tile_pool(name="wu_ps", bufs=1, space="PSUM")
            )
            wu = wu_ps.tile([P, 512], f32, tag="wu")
            for r in range(24):
                nc.tensor.matmul(
                    wu[:], lhsT=junk[:, 0:P], rhs=junk[:],
                    start=(r == 0), stop=(r == 23),
                )
            ws = consts.tile([1, 4], f32, tag="ws")
            nc.vector.tensor_copy(ws[:], wu[0:1, 0:4])
            nc.sync.dma_start(warm, ws[:])

        # ---- K/Q mb0 chunk-major over arriving x8 chunks ----
        with ExitStack() as actx:
            kq_ps = actx.enter_context(
                tc.tile_pool(name="kq_ps", bufs=1, space="PSUM")
            )
            ka = kq_ps.tile([P, S], f32, tag="ka")
            qa = kq_ps.tile([P, S], f32, tag="qa")
            for i in range(NEC // 2):
                for wt, acc in ((wk_t, ka), (wq_t, qa)):
                    for qt in range(4):
                        nc.tensor.matmul(
                            acc[:, qt * 512:(qt + 1) * 512],
                            lhsT=wt[:, i, :, 0:P],
                            rhs=xT8[:, i, :, qt * 512:(qt + 1) * 512],
                            perf_mode=DR,
                            start=(i == 0),
                            stop=(i == NEC // 2 - 1),
                        )
            # PSUM -> bf16 SBUF; DVE takes only kcT half0 (the rope
            # critical chain), ACT (idle during the prologue) the rest
            nc.scalar.copy(qcT[:, 0, 0:1024], qa[:, 0:1024])
            nc.vector.tensor_copy(kcT[:, 0, 0:1024], ka[:, 0:1024])
            nc.scalar.copy(kcT[:, 0, 1024:2048], ka[:, 1024:2048])
            nc.scalar.copy(qcT[:, 0, 1024:2048], qa[:, 1024:2048])

        shuf_mask = list(range(16, 32)) + list(range(16))
        sh_pool = ctx.enter_context(tc.tile_pool(name="sh", bufs=2))

        def rope_k_half(mb, hf):
            sl = slice(1024 * hf, 1024 * hf + 1024)
            sh = sh_pool.tile([P, 1024], bf16, tag="shk", name=f"shk{mb}_{hf}")
            nc.vector.stream_shuffle(sh[:], kcT[:, mb, sl], shuf_mask)
            nc.vector.tensor_mul(sh[:], sh[:], sin_t[:, sl])
            nc.vector.tensor_mul(kT[:, mb, sl], kcT[:, mb, sl], cos_t[:, sl])
            nc.vector.tensor_add(kT[:, mb, sl], kT[:, mb, sl], sh[:])

        def rope_q_half(mb, hf):
            sl = slice(1024 * hf, 1024 * hf + 1024)
            sh = sh_pool.tile([P, 1024], bf16, tag="shq", name=f"shq{mb}_{hf}")
            nc.vector.stream_shuffle(sh[:], qcT[:, mb, sl], shuf_mask)
            nc.vector.tensor_mul(sh[:], sh[:], sin_t[:, sl])
            qr = sh_pool.tile([P, 1024], bf16, tag="qr", name=f"qr{mb}_{hf}")
            nc.vector.tensor_mul(qr[:], qcT[:, mb, sl], cos_t[:, sl])
            nc.vector.tensor_add(qr[:], qr[:], sh[:])
            nc.vector.tensor_copy(qz[0:DH, mb, 0, sl], qr[0:DH, :])
            nc.vector.tensor_copy(qz[DH:P, mb, 1, sl], qr[DH:P, :])

        # unblock h0 pass0 ASAP: only mb0 half0 rope is on its path
        rope_k_half(0, 0)
        rope_q_half(0, 0)

        def vn_dst(sb, mb):
            return vn[:, sb, 2 * mb:2 * mb + 2, 0:64]

        # rope halves1 go on the DVE queue now, ahead of the V-phase DVE
        # work (qz half1 is needed when h0 pass1 starts)
        rope_q_half(0, 1)
        rope_k_half(0, 1)

        # ---- attention + filler machinery ----
        # Fillers are work quanta interleaved into the ACT-paced
        # attention iterations. CRITICAL: a filler that WRITES data read
        # by a later pass must be emitted (program order) before that
        # pass's reads -- Tile deps only point backward -- so the deque
        # has a hard flush point before h==2.
        fillers = deque()

        def emit_fillers(n):
            for _ in range(n):
                if not fillers:
                    return
                fillers.popleft()()

        def flush_fillers():
            while fillers:
                fillers.popleft()()

        attctx = ExitStack()
        sc_ps = attctx.enter_context(
            tc.tile_pool(name="sc_ps", bufs=2, space="PSUM")
        )
        # deep probs buffering: the first two passes run with PV fully
        # deferred (their V^T blocks arrive only at ~x^T-DMA completion),
        # so up to 16 pt tiles are alive at once
        ptp = ctx.enter_context(tc.tile_pool(name="ptp", bufs=18))
        dn = ctx.enter_context(tc.tile_pool(name="dn", bufs=2))
        pools = {}
        ycfg = {"pool": None, "tail": False}
        ys_pool = ctx.enter_context(tc.tile_pool(name="ys", bufs=4))
        vs_pool = ctx.enter_context(tc.tile_pool(name="vs", bufs=3))

        # --- filler generators (transposes, proj quarters, y halves) ---
        def mk_tp(sb, mb):
            if DMA_TP:
                def f():
                    # DMA-xbar transpose into contiguous staging, then a
                    # DVE copy into vn's 65-wide slot layout; zero PE.
                    eng = nc.sync if sb % 2 == 0 else nc.scalar
                    vs = vs_pool.tile([P, P], bf16, tag="vs",
                                      name=f"vs{mb}_{sb}")
                    eng.dma_start_transpose(
                        vs[:], vT[:, mb, sb * P:(sb + 1) * P]
                    )
                    nc.vector.tensor_copy(
                        vn_dst(sb, mb),
                        vs[:].rearrange("p (a b) -> p a b", a=2),
                    )
            else:
                def f():
                    tp = pools["yq"].tile([P, P], bf16, tag="pq",
                                          name=f"tp{mb}s_{sb}")
                    nc.tensor.transpose(
                        tp[:], vT[:, mb, sb * P:(sb + 1) * P], ident[:]
                    )
                    nc.vector.tensor_copy(
                        vn_dst(sb, mb), tp[:].rearrange("p (a b) -> p a b", a=2)
                    )
            return f

        pq_state = {}

        def make_proj_quarter(wt, dst, qi, tagname, fp8=False):
            steps = []
            nsteps = NEC // 2 if fp8 else NEC

            def mk_mm(i):
                def f():
                    if i == 0:
                        pq_state["t"] = pools["yq"].tile(
                            [P, 512], f32, tag="pq",
                            name=f"pq_{tagname}_{qi}",
                        )
                    if fp8:
                        nc.tensor.matmul(
                            pq_state["t"][:],
                            lhsT=wt[:, i, :, P:DC],
                            rhs=xT8[:, i, :, 512 * qi:512 * qi + 512],
                            perf_mode=DR,
                            start=(i == 0),
                            stop=(i == nsteps - 1),
                        )
                    else:
                        nc.tensor.matmul(
                            pq_state["t"][:],
                            lhsT=wt[:, i, P:DC],
                            rhs=xT[:, i, 512 * qi:512 * qi + 512],
                            start=(i == 0),
                            stop=(i == nsteps - 1),
                        )
                return f

            for i in range(nsteps):
                steps.append(mk_mm(i))

            def cp():
                nc.vector.tensor_copy(
                    dst[:, 1, 512 * qi:512 * qi + 512], pq_state["t"][:]
                )
            steps.append(cp)
            return steps

        mb1_steps = []
        for qi in range(4):
            mb1_steps.extend(make_proj_quarter(wk_t, kcT, qi, "k", fp8=True))
        mb1_steps.append(lambda: rope_k_half(1, 0))
        mb1_steps.append(lambda: rope_k_half(1, 1))
        for qi in range(4):
            mb1_steps.extend(make_proj_quarter(wq_t, qcT, qi, "q", fp8=True))
        mb1_steps.append(lambda: rope_q_half(1, 0))
        mb1_steps.append(lambda: rope_q_half(1, 1))
        for qi in range(4):
            mb1_steps.extend(make_proj_quarter(wv_t, vT, qi, "v"))
        for sb in range(NSB):
            mb1_steps.append(mk_tp(sb, 1))
        fillers.extend(mb1_steps)

        def v_mb0_section():
            # V mb0 + all 16 transposes, emitted AFTER the burst passes'
            # scores so the exp stream is never queued behind the
            # x^T-DMA-gated V matmuls in the PE FIFO.
            # sc_ps holds 4 banks during the burst passes, so va (4) and
            # tp (3) must be sequential, not nested
            with ExitStack() as vctx:
                v_ps = vctx.enter_context(
                    tc.tile_pool(name="v_ps", bufs=1, space="PSUM")
                )
                va = v_ps.tile([P, S], f32, tag="va")
                for i in range(NEC):
                    for qt in range(4):
                        nc.tensor.matmul(
                            va[:, qt * 512:(qt + 1) * 512],
                            lhsT=wv_t[:, i, 0:P],
                            rhs=xT[:, i, qt * 512:(qt + 1) * 512],
                            start=(i == 0),
                            stop=(i == NEC - 1),
                        )
                nc.vector.tensor_copy(vT[:, 0, 0:1024], va[:, 0:1024])
                nc.vector.tensor_copy(vT[:, 0, 1024:2048], va[:, 1024:2048])
            with ExitStack() as vctx:
                tp_ps = vctx.enter_context(
                    tc.tile_pool(name="tp_ps", bufs=3, space="PSUM")
                )
                for sb in range(NSB):
                    tp = tp_ps.tile([P, P], bf16, tag="tp", name=f"tp0_{sb}")
                    nc.tensor.transpose(
                        tp[:], vT[:, 0, sb * P:(sb + 1) * P], ident[:]
                    )
                    nc.vector.tensor_copy(
                        vn_dst(sb, 0), tp[:].rearrange("p (a b) -> p a b", a=2)
                    )

        yq_state = {}

        def mk_y_half(sb, eh):
            e0 = 512 * eh

            def q1():
                yq_state["t"] = ycfg["pool"].tile(
                    [P, 512], f32, tag="pq", name=f"yq_{sb}_{eh}"
                )
                nc.tensor.matmul(
                    yq_state["t"][:],
                    lhsT=onrm[:, 0, sb * P:(sb + 1) * P],
                    rhs=wo_t[:, 0, e0:e0 + 512],
                    start=True,
                    stop=False,
                )

            def q2():
                nc.tensor.matmul(
                    yq_state["t"][:],
                    lhsT=onrm[:, 1, sb * P:(sb + 1) * P],
                    rhs=wo_t[:, 1, e0:e0 + 512],
                    start=False,
                    stop=True,
                )
                ys = ys_pool.tile([P, 512], bf16, tag="ys",
                                  name=f"ys_{sb}_{eh}")
                if ycfg["tail"] and eh == 1:
                    # post-attention: ACT + the scalar DMA ring are free
                    nc.scalar.copy(ys[:], yq_state["t"][:])
                    nc.scalar.dma_start(
                        y[sb * P:(sb + 1) * P, e0:e0 + 512], ys[:]
                    )
                else:
                    nc.vector.tensor_copy(ys[:], yq_state["t"][:])
                    nc.sync.dma_start(
                        y[sb * P:(sb + 1) * P, e0:e0 + 512], ys[:]
                    )

            return [q1, q2]

        # --- attention passes: h-major, single head per pass ---
        # Epilogue staging: den+recip (DVE) fire at bank completion;
        # the gpsimd broadcast + DVE multiply are DEFERRED a couple of
        # iterations so they reach their FIFOs with inputs long ready
        # and never head-of-line-block the PV path.
        deferred = deque()

        def run_deferred():
            while deferred:
                deferred.popleft()()

        def attention_pass(h, pss, after_post0=None, defer_pv=False, nf=2):
            mb, par = h // 2, h % 2
            q0 = pss * 1024
            nti = 8 if pss == 0 else 16
            stopA = (q0 + 512) // P - 1
            stopB = (q0 + 1024) // P - 1
            acc_state = {}

            def get_accs():
                # acc banks are allocated lazily: for deferred-PV passes
                # the PSUM pools don't exist yet at scores/exp time
                if "a" not in acc_state:
                    acc_state["a"] = pools["accA"].tile(
                        [65, 512], f32, tag="accA", name=f"accA_{h}_{pss}"
                    )
                    acc_state["b"] = pools["accB"].tile(
                        [65, 512], f32, tag="accB", name=f"accB_{h}_{pss}"
                    )
                return acc_state["a"], acc_state["b"]

            def issue_pv(pt, w0, ti):
                accs = get_accs()
                for bk in range(2):
                    lo = max(w0, q0 + 512 * bk)
                    hi = q0 + 512 * (bk + 1)
                    if lo >= hi:
                        continue
                    b0 = q0 + 512 * bk
                    nc.tensor.matmul(
                        accs[bk][:, lo - b0:hi - b0],
                        lhsT=vn[:, ti, h, :],
                        rhs=pt[:, lo - q0:hi - q0],
                        start=(ti == 0),
                        stop=(ti == (stopA if bk == 0 else stopB)),
                    )

            def norm_pre(bk):
                acc = get_accs()[bk]
                den = dn.tile([1, 512], f32, tag="den",
                              name=f"den_{h}_{pss}_{bk}")
                nc.vector.tensor_copy(den[:], acc[64:65, :])
                rden = dn.tile([1, 512], f32, tag="rden",
                               name=f"rden_{h}_{pss}_{bk}")
                nc.vector.reciprocal_approx_fast(rden[:], den[:])
                return rden

            def norm_post(bk, rden):
                gcol = q0 + 512 * bk
                acc = get_accs()[bk]
                rdb = dn.tile([DH, 512], f32, tag="rdb",
                              name=f"rdb_{h}_{pss}_{bk}")
                nc.gpsimd.partition_broadcast(rdb[:], rden[:])
                nc.vector.tensor_mul(
                    onrm[par * DH:par * DH + DH, mb, gcol:gcol + 512],
                    acc[0:DH, :],
                    rdb[:],
                )

            def finish(pvs):
                for args in pvs:
                    issue_pv(*args)
                    if args[2] == stopA:
                        rden = norm_pre(0)

                        def post0(rden=rden):
                            norm_post(0, rden)
                            if after_post0 is not None:
                                after_post0()
                        deferred.append(post0)
                rden = norm_pre(1)
                deferred.append(lambda rden=rden: norm_post(1, rden))

            pending = None
            pvs = []
            for ti in range(nti):
                t0 = ti * P
                w0 = max(t0, q0)
                width = q0 + 1024 - w0
                diag = t0 >= q0
                sc = sc_ps.tile([P, 1024], f32, tag="sc",
                                name=f"sc_{h}_{pss}_{ti}")
                d0 = 1024 - width
                kblk = kT[:, mb, t0:t0 + P]
                if diag:
                    # additive causal mask: preload MASK_VAL above the
                    # diagonal into the diag 128-col piece (identity-
                    # transpose matmul), then accumulate scores onto it.
                    nc.tensor.matmul(
                        sc[:, d0:d0 + P], lhsT=ident[:], rhs=msk_t[:],
                        start=True, stop=False,
                    )
                    nc.tensor.matmul(
                        sc[:, d0:d0 + P],
                        lhsT=kblk,
                        rhs=qz[:, mb, par, q0 + d0:q0 + d0 + P],
                        start=False, stop=True,
                    )
                    p0 = d0 + P
                else:
                    p0 = d0
                while p0 < 1024:
                    p1 = min(1024, (p0 // 512 + 1) * 512)
                    nc.tensor.matmul(
                        sc[:, p0:p1],
                        lhsT=kblk,
                        rhs=qz[:, mb, par, q0 + p0:q0 + p1],
                    )
                    p0 = p1
                if not defer_pv:
                    emit_fillers(nf)
                    if deferred:
                        deferred.popleft()()
                pt = ptp.tile([P, 1024], bf16, tag="pt",
                              name=f"pt_{h}_{pss}_{ti}")
                nc.scalar.activation(
                    pt[:, d0:1024], sc[:, d0:1024], AF.Exp, scale=EXP_SCALE
                )
                if defer_pv:
                    pvs.append((pt, w0, ti))
                    continue
                if pending is not None:
                    issue_pv(*pending)
                    if pending[2] == stopA:
                        rden = norm_pre(0)

                        def post0(rden=rden):
                            norm_post(0, rden)
                            if after_post0 is not None:
                                after_post0()
                        deferred.append(post0)
                pending = (pt, w0, ti)
            if defer_pv:
                return lambda: finish(pvs)
            issue_pv(*pending)
            rden = norm_pre(1)
            deferred.append(lambda rden=rden: norm_post(1, rden))
            emit_fillers(2)
            return None

        def y_after_stopA():
            for sb in range(8, 12):
                for eh in range(2):
                    fillers.extend(mk_y_half(sb, eh))

        # --- pass sequence ---
        # h0/h1 pass0 run first with PV fully deferred: their exps need
        # only kT/qz mb0 (fp8 x path, ready ~18us), while V mb0 waits on
        # the 4MB bf16 x^T DMA (~35us). The V section + PV drains slot in
        # behind those 16 score matmuls on the PE FIFO; the ACT exp
        # stream never queues behind the DMA-gated V work.
        d00 = attention_pass(0, 0, defer_pv=True)
        d10 = attention_pass(1, 0, defer_pv=True)
        v_mb0_section()
        pools["accA"] = attctx.enter_context(
            tc.tile_pool(name="accA_ps", bufs=1, space="PSUM")
        )
        pools["accB"] = attctx.enter_context(
            tc.tile_pool(name="accB_ps", bufs=2, space="PSUM")
        )
        pools["yq"] = attctx.enter_context(
            tc.tile_pool(name="yq_ps", bufs=1, space="PSUM")
        )
        ycfg["pool"] = pools["yq"]
        d00()
        # h0p0's deferred norm_posts must be EMITTED before h1p0's PV
        # drain reuses the accA bank (Tile deps only point backward)
        run_deferred()
        d10()
        attention_pass(0, 1, nf=3)
        attention_pass(1, 1, nf=3)
        # everything heads 2/3 read (kT/qz/vn mb1) must be emitted
        # before their passes' reads
        flush_fillers()
        attention_pass(2, 0)
        attention_pass(2, 1)
        attention_pass(3, 0)
        run_deferred()
        # only sb0-3 drain in-pass: the single yq PSUM bank serializes a
        # y quantum at ~1.6us, so more would block h3p1's own scores
        for sb in range(4):
            for eh in range(2):
                fillers.extend(mk_y_half(sb, eh))
        attention_pass(3, 1)
        # the deferred final norm_post must be emitted while the acc
        # pools are still open (their banks are reused by the tail pool)
        run_deferred()
        # tail: attention PSUM pools close; the remaining W_o blocks
        # drain through an 8-deep PSUM pool so the matmul->copy->DMA
        # chains pipeline instead of serializing on one bank
        attctx.close()
        tail_ps = ctx.enter_context(
            tc.tile_pool(name="tail_ps", bufs=8, space="PSUM")
        )
        ycfg["pool"] = tail_ps
        ycfg["tail"] = True
        for sb in range(4, NSB):
            for eh in range(2):
                fillers.extend(mk_y_half(sb, eh))
        flush_fillers()

        if debug:
            nc.sync.dma_start(dbg["dkT"], kT[:])
            nc.sync.dma_start(dbg["dqz"], qz[:])
            nc.sync.dma_start(dbg["dvT"], vT[:])
            nc.sync.dma_start(dbg["dvn"], vn[:])
            nc.sync.dma_start(dbg["donrm"], onrm[:])

    nc.compile()
    return nc


def get_program():
    global _PROG
    if _PROG is None:
        _PROG = _build_program()
    return _PROG


def make_in_maps(x, W_q, W_k, W_v, W_o):
    perm = _perm64()
    idx_local = (np.arange(DC) // 64) * 64 + perm[np.arange(DC) % 64]
    ang, sgn = _cos_sin_tiles()
    cos_np = np.cos(ang).astype(BF16)
    sin_np = (sgn * np.sin(ang)).astype(BF16)
    # scores tile is (t, q): additive causal mask, 0 where t <= q
    # (keep), MASK_VAL where t > q (exp -> 0)
    cmask_np = np.where(np.triu(np.ones((P, P))) > 0, 0.0,
                        MASK_VAL).astype(BF16)

    def pair8(a):
        # [E, M] -> [128, 4, 2, M] fp8 DoubleRow pair layout
        e, m = a.shape
        return np.ascontiguousarray(
            np.clip(a, -448, 448).reshape(4, 2, P, m).transpose(2, 0, 1, 3)
        ).astype(F8)

    in_maps = []
    for c in range(NCORES):
        b, hg = c // 4, c % 4
        base = hg * DC
        xt = x[b].T
        in_maps.append(
            dict(
                xbt=np.ascontiguousarray(xt.astype(BF16)),
                xbt8=pair8(xt),
                wq8=pair8(W_q[:, base + idx_local] * QK_SCALE),
                wk8=pair8(W_k[:, base + idx_local] * QK_SCALE),
                wv=np.ascontiguousarray(W_v[:, base:base + DC].astype(BF16)),
                wo=np.ascontiguousarray(W_o[base:base + DC, :].astype(BF16)),
                cosr=cos_np,
                sinr=sin_np,
                cmask=cmask_np,
            )
        )
    return in_maps


def kernel(x, W_q, W_k, W_v, W_o, _trace=False, _trace_cores=None):
    from concourse.bass_utils import run_bass_kernel_spmd

    x = np.asarray(x, dtype=np.float32)
    W_q = np.asarray(W_q, dtype=np.float32)
    W_k = np.asarray(W_k, dtype=np.float32)
    W_v = np.asarray(W_v, dtype=np.float32)
    W_o = np.asarray(W_o, dtype=np.float32)

    nc = get_program()
    in_maps = make_in_maps(x, W_q, W_k, W_v, W_o)
    res = run_bass_kernel_spmd(
        nc,
        in_maps,
        list(range(NCORES)),
        trace=_trace,
        trace_cores=_trace_cores,
    )
    y = np.zeros((B, S, E), np.float32)
    for c in range(NCORES):
        y[c // 4] += np.asarray(res.results[c]["y"], dtype=np.float32)
    if _trace:
        return y, res
    return y


# revision 37
# speedup vs baseline: 1.1999x; 1.0212x over previous
"""Multi-head causal attention with RoPE on 8 Trainium2 NeuronCores.

Sharding: data-parallel over batch (B=2) x tensor-parallel over heads
(16 heads -> 4 groups of 4). Core c handles batch c//4, heads
[(c%4)*4, (c%4)*4+4). Each core computes a partial y = attn_out @ W_o
for its head group; the host sums the 4 partials per batch (the "W_o
all-reduce").

v4 design (v2 dtypes + early-start scheduling):
  - Q/K projections in fp8 DoubleRow from an fp8 x^T copy, with the
    fp8 weights pre-scaled x16 (escapes fp8e4m3 subnormals; the exp
    scale absorbs 1/256). The value path (V, probs, o, W_o) stays bf16:
    fp8 noise in a dot-product operand does NOT average down, and
    value-path fp8 measured ~6% output error.
  - Junk warm-up matmuls at t=0 hold the PE HAM clock gate open through
    the input-DMA window, and the prologue is restructured to unblock
    the first exp at ~14us (vs ~51us): K/Q mb0 chunk-major over the
    arriving fp8 x chunks -> rope half0 immediately; V mb0 chunk-major
    over the arriving bf16 x chunks; only transposes sb0-7 run before
    attention (sb8-15 + all of mb1 are fillers).
  - V-block transposes for the filler phase go through the DMA xbar
    (dma_start_transpose) instead of TensorE, freeing ~6us of PE time
    in the attention phase where PE is the co-bottleneck with ACT.
  - Attention: one head per pass, q in 1024-col passes, scores^T[t,q]
    as a single matmul per t-block, exp on ACT (the pass pacer), PV
    software-pipelined one iteration behind, denominator as a 65th
    ones-column of V. Causal mask preloaded additively (-1e7) via an
    identity-transpose matmul so the exp->PV path stays clean.
  - Epilogue per 512-col PSUM bank: den copy -> reciprocal_approx_fast
    -> gpsimd partition_broadcast -> one DVE multiply from PSUM.
  - y (o @ W_o) quanta interleave as fillers: sb0-7 during the last
    head's second pass, sb8-11 appended mid-pass once that pass's
    bank-A epilogue lands, so the tail only drains sb12-15.
"""

import os
import sys
from collections import deque
from contextlib import ExitStack

import numpy as np

for _p in ("/opt/trn_rl_repo",):
    if os.path.isdir(_p) and _p not in sys.path:
        sys.path.insert(0, _p)

import ml_dtypes  # noqa: E402

BF16 = ml_dtypes.bfloat16
F8 = ml_dtypes.float8_e4m3fn

B, S, E = 2, 2048, 1024
H, DH = 16, 64
NCORES = 8
HPC = H // 4          # 4 heads per core
DC = HPC * DH         # 256 head dims per core
ATTN_SCALE = 1.0 / 32.0  # 1/sqrt(E)
ROPE_BASE = 10000.0
P = 128
NSB = S // P          # 16 sequence blocks
NEC = E // P          # 8 E chunks
MB = DC // P          # 2 partition blocks of head dims

QK_SCALE = 16.0       # host pre-scale on W_q/W_k (fp8 subnormal escape)
EXP_SCALE = ATTN_SCALE / (QK_SCALE * QK_SCALE)
MASK_VAL = -1e7       # additive causal mask; * EXP_SCALE -> exp underflows to 0

DMA_TP = os.environ.get("KNL_DMATP", "1") == "1"

_PROG = None


def _perm64():
    """perm[j] = original head-dim index stored at permuted position j.

    Quadrant q of the permuted layout holds RoPE pairs i in
    [16q, 16q+16): even elements (2i) at slots 0-15, odd (2i+1) at
    slots 16-31. The rotation partner is then always +-16 partitions
    away within one 32-partition quadrant (stream_shuffle range).
    """
    j = np.arange(64)
    qd, r = j // 32, j % 32
    i = 16 * qd + (r % 16)
    return 2 * i + (r >= 16)


def _cos_sin_tiles():
    pl = np.arange(P) % 64
    qd, r = pl // 32, pl % 32
    i = 16 * qd + (r % 16)
    inv = ROPE_BASE ** (-(2.0 * i) / DH)
    ang = np.arange(S)[None, :] * inv[:, None]          # (128, S)
    sgn = np.where(r < 16, -1.0, 1.0)[:, None]
    return ang, sgn


def _build_program(debug=False):
    import concourse.bacc as bacc
    import concourse.tile as tile
    from concourse import masks, mybir

    f32 = mybir.dt.float32
    bf16 = mybir.dt.bfloat16
    f8 = mybir.dt.float8e4
    AF = mybir.ActivationFunctionType
    DR = mybir.MatmulPerfMode.DoubleRow

    nc = bacc.Bacc("TRN2", target_bir_lowering=False, debug=False)
    xbt = nc.dram_tensor("xbt", [E, S], bf16, kind="ExternalInput").ap()
    # fp8 copies of x^T / W_q / W_k in DoubleRow pair layout
    # [p, chunk-pair i, j, *]: element (2i+j)*128+p of the E axis
    xbt8 = nc.dram_tensor("xbt8", [P, NEC // 2, 2, S], f8,
                          kind="ExternalInput").ap()
    wq8 = nc.dram_tensor("wq8", [P, NEC // 2, 2, DC], f8,
                         kind="ExternalInput").ap()
    wk8 = nc.dram_tensor("wk8", [P, NEC // 2, 2, DC], f8,
                         kind="ExternalInput").ap()
    wv = nc.dram_tensor("wv", [E, DC], bf16, kind="ExternalInput").ap()
    wo = nc.dram_tensor("wo", [DC, E], bf16, kind="ExternalInput").ap()
    cosr = nc.dram_tensor("cosr", [P, S], bf16, kind="ExternalInput").ap()
    sinr = nc.dram_tensor("sinr", [P, S], bf16, kind="ExternalInput").ap()
    cmask = nc.dram_tensor("cmask", [P, P], bf16, kind="ExternalInput").ap()
    y = nc.dram_tensor("y", [S, E], bf16, kind="ExternalOutput").ap()
    # tiny output read from the warm-up accumulator so neuronxcc cannot
    # dead-code-eliminate the junk matmuls that keep the HAM clock warm
    warm = nc.dram_tensor("warm", [1, 4], f32, kind="ExternalOutput").ap()
    dbg = {}
    if debug:
        for nm, shp in (
            ("dkT", [P, MB, S]), ("dqz", [P, MB, 2, S]), ("dvT", [P, MB, S]),
            ("dvn", [P, NSB, HPC, 65]), ("donrm", [P, MB, S]),
        ):
            dbg[nm] = nc.dram_tensor(nm, shp, bf16, kind="ExternalOutput").ap()

    with ExitStack() as ctx:
        tc = ctx.enter_context(tile.TileContext(nc))
        consts = ctx.enter_context(tc.tile_pool(name="consts", bufs=1))
        persist = ctx.enter_context(tc.tile_pool(name="persist", bufs=1))

        ident = consts.tile([P, P], bf16, tag="ident")
        junk = consts.tile([P, 512], bf16, tag="junk")
        wk_t = consts.tile([P, NEC // 2, 2, DC], f8, tag="wk")
        wq_t = consts.tile([P, NEC // 2, 2, DC], f8, tag="wq")
        wv_t = consts.tile([P, NEC, DC], bf16, tag="wv")
        wo_t = consts.tile([P, MB, E], bf16, tag="wo")
        cos_t = consts.tile([P, S], bf16, tag="cos")
        sin_t = consts.tile([P, S], bf16, tag="sin")
        msk_t = consts.tile([P, P], bf16, tag="msk")

        xT = persist.tile([P, NEC, S], bf16, tag="xT")
        xT8 = persist.tile([P, NEC // 2, 2, S], f8, tag="xT8")
        kcT = persist.tile([P, MB, S], bf16, tag="kcT")
        qcT = persist.tile([P, MB, S], bf16, tag="qcT")
        vT = persist.tile([P, MB, S], bf16, tag="vT")
        # RoPE'd K^T, and Q^T zero-padded per head parity: slice
        # [:, mb, par, :] has head (2*mb+par)'s 64 rows live and the
        # other 64 rows zero, so scores use the FULL 128-row K^T block
        # as lhsT (the HAM clock gate never grants full clock to
        # partial-height matmul streams; zero rows contribute 0).
        kT = persist.tile([P, MB, S], bf16, tag="kT")
        qz = persist.tile([P, MB, 2, S], bf16, tag="qz")
        vn = persist.tile([P, NSB, HPC, 65], bf16, tag="vn")
        onrm = persist.tile([P, MB, S], bf16, tag="onrm")

        # ---- t=0 setup: memsets, masks, warm-up, DMA streams ----
        nc.vector.memset(junk[:], 1.0)
        nc.vector.memset(vn[:, :, :, 64:65], 1.0)
        nc.gpsimd.memset(qz[0:DH, :, 1, :], 0.0)
        nc.gpsimd.memset(qz[DH:P, :, 0, :], 0.0)
        masks.make_identity(nc, ident[:])

        wv_r = wv.rearrange("(c p) m -> p c m", p=P)
        # The rings sustain ~130 GB/s each, so ordering is everything:
        # x8 (gates the exp stream via K/Q+rope) goes first on both
        # rings, then the rope tables, then the 4MB bf16 x^T (gates only
        # the deferred-PV value path), then late weights.
        # sync ring: wk8, x8 evens, sin h0, wvA(mb0), xbt evens, sin h1, wvB
        # scalar ring: wq8, x8 odds, cos h0, cmask, xbt odds, cos h1, wo
        nc.sync.dma_start(wk_t[:], wk8)
        nc.scalar.dma_start(wq_t[:], wq8)
        for ec in range(NEC // 2):
            eng = nc.sync if ec % 2 == 0 else nc.scalar
            eng.dma_start(xT8[:, ec, :, :], xbt8[:, ec, :, :])
        nc.sync.dma_start(sin_t[:, 0:1024], sinr[:, 0:1024])
        nc.scalar.dma_start(cos_t[:, 0:1024], cosr[:, 0:1024])
        nc.scalar.dma_start(msk_t[:], cmask)
        nc.sync.dma_start(wv_t[:, :, 0:P], wv_r[:, :, 0:P])
        for ec in range(NEC):
            eng = nc.sync if ec % 2 == 0 else nc.scalar
            eng.dma_start(xT[:, ec, :], xbt[ec * P:(ec + 1) * P, :])
            if ec == 1:
                nc.sync.dma_start(sin_t[:, 1024:2048], sinr[:, 1024:2048])
                nc.scalar.dma_start(cos_t[:, 1024:2048], cosr[:, 1024:2048])
        nc.sync.dma_start(wv_t[:, :, P:DC], wv_r[:, :, P:DC])
        nc.scalar.dma_start(wo_t[:], wo.rearrange("(c p) n -> p c n", p=P))

        # exp spline table preload (one-time ~1.3us) off the critical path
        scr = consts.tile([P, 16], f32, tag="scr")
        nc.scalar.activation(scr[:], junk[:, 0:16], AF.Exp, scale=EXP_SCALE)
        # gpsimd ANT-lib preload: partition_broadcast's custom library
        # loads once here (~9us, hidden under input DMA)
        scr2 = consts.tile([16, 16], f32, tag="scr2")
        scr3 = consts.tile([1, 16], f32, tag="scr3")
        nc.vector.memset(scr3[:], 1.0)
        nc.gpsimd.partition_broadcast(scr2[:], scr3[:])

        # warm-up matmuls: the PE must stay busy from t=0 until the first
        # x8 chunk lands (~13us: ~6us queue bootstrap + transfer), else
        # the HAM MID window re-throttles the clock to 1.2 GHz for the
        # whole prologue. One ACCUMULATION CHAIN whose result is DMA'd
        # to a tiny output, so neuronxcc cannot dead-code-eliminate it
        # (independent dead-write matmuls get pruned).
        with ExitStack() as wctx:
            wu_ps = wctx.enter_context(
                tc.tile_pool(name="wu_ps", bufs=1, space="PSUM")
            )
            wu = wu_ps.tile([P, 512], f32, tag="wu")
            for r in range(24):
                nc.tensor.matmul(
                    wu[:], lhsT=junk[:, 0:P], rhs=junk[:],
                    start=(r == 0), stop=(r == 23),
                )
            ws = consts.tile([1, 4], f32, tag="ws")
            nc.vector.tensor_copy(ws[:], wu[0:1, 0:4])
            nc.sync.dma_start(warm, ws[:])

        # ---- K/Q mb0 chunk-major over arriving x8 chunks ----
        with ExitStack() as actx:
            kq_ps = actx.enter_context(
                tc.tile_pool(name="kq_ps", bufs=1, space="PSUM")
            )
            ka = kq_ps.tile([P, S], f32, tag="ka")
            qa = kq_ps.tile([P, S], f32, tag="qa")
            for i in range(NEC // 2):
                for wt, acc in ((wk_t, ka), (wq_t, qa)):
                    for qt in range(4):
                        nc.tensor.matmul(
                            acc[:, qt * 512:(qt + 1) * 512],
                            lhsT=wt[:, i, :, 0:P],
                            rhs=xT8[:, i, :, qt * 512:(qt + 1) * 512],
                            perf_mode=DR,
                            start=(i == 0),
                            stop=(i == NEC // 2 - 1),
                        )
            # PSUM -> bf16 SBUF; DVE takes only kcT half0 (the rope
            # critical chain), ACT (idle during the prologue) the rest
            nc.scalar.copy(qcT[:, 0, 0:1024], qa[:, 0:1024])
            nc.vector.tensor_copy(kcT[:, 0, 0:1024], ka[:, 0:1024])
            nc.scalar.copy(kcT[:, 0, 1024:2048], ka[:, 1024:2048])
            nc.scalar.copy(qcT[:, 0, 1024:2048], qa[:, 1024:2048])

        shuf_mask = list(range(16, 32)) + list(range(16))
        sh_pool = ctx.enter_context(tc.tile_pool(name="sh", bufs=2))

        def rope_k_half(mb, hf):
            sl = slice(1024 * hf, 1024 * hf + 1024)
            sh = sh_pool.tile([P, 1024], bf16, tag="shk", name=f"shk{mb}_{hf}")
            nc.vector.stream_shuffle(sh[:], kcT[:, mb, sl], shuf_mask)
            nc.vector.tensor_mul(sh[:], sh[:], sin_t[:, sl])
            nc.vector.tensor_mul(kT[:, mb, sl], kcT[:, mb, sl], cos_t[:, sl])
            nc.vector.tensor_add(kT[:, mb, sl], kT[:, mb, sl], sh[:])

        def rope_q_half(mb, hf):
            sl = slice(1024 * hf, 1024 * hf + 1024)
            sh = sh_pool.tile([P, 1024], bf16, tag="shq", name=f"shq{mb}_{hf}")
            nc.vector.stream_shuffle(sh[:], qcT[:, mb, sl], shuf_mask)
            nc.vector.tensor_mul(sh[:], sh[:], sin_t[:, sl])
            qr = sh_pool.tile([P, 1024], bf16, tag="qr", name=f"qr{mb}_{hf}")
            nc.vector.tensor_mul(qr[:], qcT[:, mb, sl], cos_t[:, sl])
            nc.vector.tensor_add(qr[:], qr[:], sh[:])
            nc.vector.tensor_copy(qz[0:DH, mb, 0, sl], qr[0:DH, :])
            nc.vector.tensor_copy(qz[DH:P, mb, 1, sl], qr[DH:P, :])

        # unblock h0 pass0 ASAP: only mb0 half0 rope is on its path
        rope_k_half(0, 0)
        rope_q_half(0, 0)

        def vn_dst(sb, mb):
            return vn[:, sb, 2 * mb:2 * mb + 2, 0:64]

        # rope halves1 go on the DVE queue now, ahead of the V-phase DVE
        # work (qz half1 is needed when h0 pass1 starts)
        rope_q_half(0, 1)
        rope_k_half(0, 1)

        # ---- attention + filler machinery ----
        # Fillers are work quanta interleaved into the ACT-paced
        # attention iterations. CRITICAL: a filler that WRITES data read
        # by a later pass must be emitted (program order) before that
        # pass's reads -- Tile deps only point backward -- so the deque
        # has a hard flush point before h==2.
        fillers = deque()

        def emit_fillers(n):
            for _ in range(n):
                if not fillers:
                    return
                fillers.popleft()()

        def flush_fillers():
            while fillers:
                fillers.popleft()()

        attctx = ExitStack()
        sc_ps = attctx.enter_context(
            tc.tile_pool(name="sc_ps", bufs=2, space="PSUM")
        )
        # deep probs buffering: the first two passes run with PV fully
        # deferred (their V^T blocks arrive only at ~x^T-DMA completion),
        # so up to 16 pt tiles are alive at once
        ptp = ctx.enter_context(tc.tile_pool(name="ptp", bufs=18))
        dn = ctx.enter_context(tc.tile_pool(name="dn", bufs=2))
        pools = {}
        ycfg = {"pool": None, "tail": False}
        ys_pool = ctx.enter_context(tc.tile_pool(name="ys", bufs=4))
        vs_pool = ctx.enter_context(tc.tile_pool(name="vs", bufs=3))

        # --- filler generators (transposes, proj quarters, y halves) ---
        def mk_tp(sb, mb):
            if DMA_TP:
                def f():
                    # DMA-xbar transpose into contiguous staging, then a
                    # DVE copy into vn's 65-wide slot layout; zero PE.
                    eng = nc.sync if sb % 2 == 0 else nc.scalar
                    vs = vs_pool.tile([P, P], bf16, tag="vs",
                                      name=f"vs{mb}_{sb}")
                    eng.dma_start_transpose(
                        vs[:], vT[:, mb, sb * P:(sb + 1) * P]
                    )
                    nc.vector.tensor_copy(
                        vn_dst(sb, mb),
                        vs[:].rearrange("p (a b) -> p a b", a=2),
                    )
            else:
                def f():
                    tp = pools["yq"].tile([P, P], bf16, tag="pq",
                                          name=f"tp{mb}s_{sb}")
                    nc.tensor.transpose(
                        tp[:], vT[:, mb, sb * P:(sb + 1) * P], ident[:]
                    )
                    nc.vector.tensor_copy(
                        vn_dst(sb, mb), tp[:].rearrange("p (a b) -> p a b", a=2)
                    )
            return f

        pq_state = {}

        def make_proj_quarter(wt, dst, qi, tagname, fp8=False):
            steps = []
            nsteps = NEC // 2 if fp8 else NEC

            def mk_mm(i):
                def f():
                    if i == 0:
                        pq_state["t"] = pools["yq"].tile(
                            [P, 512], f32, tag="pq",
                            name=f"pq_{tagname}_{qi}",
                        )
                    if fp8:
                        nc.tensor.matmul(
                            pq_state["t"][:],
                            lhsT=wt[:, i, :, P:DC],
                            rhs=xT8[:, i, :, 512 * qi:512 * qi + 512],
                            perf_mode=DR,
                            start=(i == 0),
                            stop=(i == nsteps - 1),
                        )
                    else:
                        nc.tensor.matmul(
                            pq_state["t"][:],
                            lhsT=wt[:, i, P:DC],
                            rhs=xT[:, i, 512 * qi:512 * qi + 512],
                            start=(i == 0),
                            stop=(i == nsteps - 1),
                        )
                return f

            for i in range(nsteps):
                steps.append(mk_mm(i))

            def cp():
                nc.vector.tensor_copy(
                    dst[:, 1, 512 * qi:512 * qi + 512], pq_state["t"][:]
                )
            steps.append(cp)
            return steps

        mb1_steps = []
        for qi in range(4):
            mb1_steps.extend(make_proj_quarter(wk_t, kcT, qi, "k", fp8=True))
        mb1_steps.append(lambda: rope_k_half(1, 0))
        mb1_steps.append(lambda: rope_k_half(1, 1))
        for qi in range(4):
            mb1_steps.extend(make_proj_quarter(wq_t, qcT, qi, "q", fp8=True))
        mb1_steps.append(lambda: rope_q_half(1, 0))
        mb1_steps.append(lambda: rope_q_half(1, 1))
        for qi in range(4):
            mb1_steps.extend(make_proj_quarter(wv_t, vT, qi, "v"))
        for sb in range(NSB):
            mb1_steps.append(mk_tp(sb, 1))
        fillers.extend(mb1_steps)

        def v_mb0_section():
            # V mb0 + all 16 transposes, emitted AFTER the burst passes'
            # scores so the exp stream is never queued behind the
            # x^T-DMA-gated V matmuls in the PE FIFO.
            # sc_ps holds 4 banks during the burst passes, so va (4) and
            # tp (3) must be sequential, not nested
            with ExitStack() as vctx:
                v_ps = vctx.enter_context(
                    tc.tile_pool(name="v_ps", bufs=1, space="PSUM")
                )
                va = v_ps.tile([P, S], f32, tag="va")
                for i in range(NEC):
                    for qt in range(4):
                        nc.tensor.matmul(
                            va[:, qt * 512:(qt + 1) * 512],
                            lhsT=wv_t[:, i, 0:P],
                            rhs=xT[:, i, qt * 512:(qt + 1) * 512],
                            start=(i == 0),
                            stop=(i == NEC - 1),
                        )
                nc.vector.tensor_copy(vT[:, 0, 0:1024], va[:, 0:1024])
                nc.vector.tensor_copy(vT[:, 0, 1024:2048], va[:, 1024:2048])
            with ExitStack() as vctx:
                tp_ps = vctx.enter_context(
                    tc.tile_pool(name="tp_ps", bufs=3, space="PSUM")
                )
                for sb in range(NSB):
                    tp = tp_ps.tile([P, P], bf16, tag="tp", name=f"tp0_{sb}")
                    nc.tensor.transpose(
                        tp[:], vT[:, 0, sb * P:(sb + 1) * P], ident[:]
                    )
                    nc.vector.tensor_copy(
                        vn_dst(sb, 0), tp[:].rearrange("p (a b) -> p a b", a=2)
                    )

        yq_state = {}

        def mk_y_half(sb, eh):
            e0 = 512 * eh

            def q1():
                yq_state["t"] = ycfg["pool"].tile(
                    [P, 512], f32, tag="pq", name=f"yq_{sb}_{eh}"
                )
                nc.tensor.matmul(
                    yq_state["t"][:],
                    lhsT=onrm[:, 0, sb * P:(sb + 1) * P],
                    rhs=wo_t[:, 0, e0:e0 + 512],
                    start=True,
                    stop=False,
                )

            def q2():
                nc.tensor.matmul(
                    yq_state["t"][:],
                    lhsT=onrm[:, 1, sb * P:(sb + 1) * P],
                    rhs=wo_t[:, 1, e0:e0 + 512],
                    start=False,
                    stop=True,
                )
                ys = ys_pool.tile([P, 512], bf16, tag="ys",
                                  name=f"ys_{sb}_{eh}")
                if ycfg["tail"] and eh == 1:
                    # post-attention: ACT + the scalar DMA ring are free
                    nc.scalar.copy(ys[:], yq_state["t"][:])
                    nc.scalar.dma_start(
                        y[sb * P:(sb + 1) * P, e0:e0 + 512], ys[:]
                    )
                else:
                    nc.vector.tensor_copy(ys[:], yq_state["t"][:])
                    nc.sync.dma_start(
                        y[sb * P:(sb + 1) * P, e0:e0 + 512], ys[:]
                    )

            return [q1, q2]

        # --- attention passes: h-major, single head per pass ---
        # Epilogue staging: den+recip (DVE) fire at bank completion;
        # the gpsimd broadcast + DVE multiply are DEFERRED a couple of
        # iterations so they reach their FIFOs with inputs long ready
        # and never head-of-line-block the PV path.
        deferred = deque()

        def run_deferred():
            while deferred:
                deferred.popleft()()

        def attention_pass(h, pss, after_post0=None, defer_pv=False, nf=2):
            mb, par = h // 2, h % 2
            q0 = pss * 1024
            nti = 8 if pss == 0 else 16
            stopA = (q0 + 512) // P - 1
            stopB = (q0 + 1024) // P - 1
            acc_state = {}

            def get_accs():
                # acc banks are allocated lazily: for deferred-PV passes
                # the PSUM pools don't exist yet at scores/exp time
                if "a" not in acc_state:
                    acc_state["a"] = pools["accA"].tile(
                        [65, 512], f32, tag="accA", name=f"accA_{h}_{pss}"
                    )
                    acc_state["b"] = pools["accB"].tile(
                        [65, 512], f32, tag="accB", name=f"accB_{h}_{pss}"
                    )
                return acc_state["a"], acc_state["b"]

            def issue_pv(pt, w0, ti):
                accs = get_accs()
                for bk in range(2):
                    lo = max(w0, q0 + 512 * bk)
                    hi = q0 + 512 * (bk + 1)
                    if lo >= hi:
                        continue
                    b0 = q0 + 512 * bk
                    nc.tensor.matmul(
                        accs[bk][:, lo - b0:hi - b0],
                        lhsT=vn[:, ti, h, :],
                        rhs=pt[:, lo - q0:hi - q0],
                        start=(ti == 0),
                        stop=(ti == (stopA if bk == 0 else stopB)),
                    )

            def norm_pre(bk):
                acc = get_accs()[bk]
                den = dn.tile([1, 512], f32, tag="den",
                              name=f"den_{h}_{pss}_{bk}")
                nc.vector.tensor_copy(den[:], acc[64:65, :])
                rden = dn.tile([1, 512], f32, tag="rden",
                               name=f"rden_{h}_{pss}_{bk}")
                nc.vector.reciprocal_approx_fast(rden[:], den[:])
                return rden

            def norm_post(bk, rden):
                gcol = q0 + 512 * bk
                acc = get_accs()[bk]
                rdb = dn.tile([DH, 512], f32, tag="rdb",
                              name=f"rdb_{h}_{pss}_{bk}")
                nc.gpsimd.partition_broadcast(rdb[:], rden[:])
                nc.vector.tensor_mul(
                    onrm[par * DH:par * DH + DH, mb, gcol:gcol + 512],
                    acc[0:DH, :],
                    rdb[:],
                )

            def finish(pvs):
                for args in pvs:
                    issue_pv(*args)
                    if args[2] == stopA:
                        rden = norm_pre(0)

                        def post0(rden=rden):
                            norm_post(0, rden)
                            if after_post0 is not None:
                                after_post0()
                        deferred.append(post0)
                rden = norm_pre(1)
                deferred.append(lambda rden=rden: norm_post(1, rden))

            pending = None
            pvs = []
            for ti in range(nti):
                t0 = ti * P
                w0 = max(t0, q0)
                width = q0 + 1024 - w0
                diag = t0 >= q0
                sc = sc_ps.tile([P, 1024], f32, tag="sc",
                                name=f"sc_{h}_{pss}_{ti}")
                d0 = 1024 - width
                kblk = kT[:, mb, t0:t0 + P]
                if diag:
                    # additive causal mask: preload MASK_VAL above the
                    # diagonal into the diag 128-col piece (identity-
                    # transpose matmul), then accumulate scores onto it.
                    nc.tensor.matmul(
                        sc[:, d0:d0 + P], lhsT=ident[:], rhs=msk_t[:],
                        start=True, stop=False,
                    )
                    nc.tensor.matmul(
                        sc[:, d0:d0 + P],
                        lhsT=kblk,
                        rhs=qz[:, mb, par, q0 + d0:q0 + d0 + P],
                        start=False, stop=True,
                    )
                    p0 = d0 + P
                else:
                    p0 = d0
                while p0 < 1024:
                    p1 = min(1024, (p0 // 512 + 1) * 512)
                    nc.tensor.matmul(
                        sc[:, p0:p1],
                        lhsT=kblk,
                        rhs=qz[:, mb, par, q0 + p0:q0 + p1],
                    )
                    p0 = p1
                if not defer_pv:
                    emit_fillers(nf)
                    if deferred:
                        deferred.popleft()()
                pt = ptp.tile([P, 1024], bf16, tag="pt",
                              name=f"pt_{h}_{pss}_{ti}")
                nc.scalar.activation(
                    pt[:, d0:1024], sc[:, d0:1024], AF.Exp, scale=EXP_SCALE
                )
                if defer_pv:
                    pvs.append((pt, w0, ti))
                    continue
                if pending is not None:
                    issue_pv(*pending)
                    if pending[2] == stopA:
                        rden = norm_pre(0)

                        def post0(rden=rden):
                            norm_post(0, rden)
                            if after_post0 is not None:
                                after_post0()
                        deferred.append(post0)
                pending = (pt, w0, ti)
            if defer_pv:
                return lambda: finish(pvs)
            issue_pv(*pending)
            rden = norm_pre(1)
            deferred.append(lambda rden=rden: norm_post(1, rden))
            emit_fillers(2)
            return None

        def y_after_stopA():
            for sb in range(8, 12):
                for eh in range(2):
                    fillers.extend(mk_y_half(sb, eh))

        # --- pass sequence ---
        # h0/h1 pass0 run first with PV fully deferred: their exps need
        # only kT/qz mb0 (fp8 x path, ready ~18us), while V mb0 waits on
        # the 4MB bf16 x^T DMA (~35us). The V section + PV drains slot in
        # behind those 16 score matmuls on the PE FIFO; the ACT exp
        # stream never queues behind the DMA-gated V work.
        d00 = attention_pass(0, 0, defer_pv=True)
        d10 = attention_pass(1, 0, defer_pv=True)
        v_mb0_section()
        pools["accA"] = attctx.enter_context(
            tc.tile_pool(name="accA_ps", bufs=1, space="PSUM")
        )
        pools["accB"] = attctx.enter_context(
            tc.tile_pool(name="accB_ps", bufs=2, space="PSUM")
        )
        pools["yq"] = attctx.enter_context(
            tc.tile_pool(name="yq_ps", bufs=1, space="PSUM")
        )
        ycfg["pool"] = pools["yq"]
        d00()
        # h0p0's deferred norm_posts must be EMITTED before h1p0's PV
        # drain reuses the accA bank (Tile deps only point backward)
        run_deferred()
        d10()
        attention_pass(0, 1, nf=3)
        attention_pass(1, 1, nf=3)
        # everything heads 2/3 read (kT/qz/vn mb1) must be emitted
        # before their passes' reads
        flush_fillers()
        attention_pass(2, 0)
        attention_pass(2, 1)
        attention_pass(3, 0)
        run_deferred()
        for sb in range(8):
            for eh in range(2):
                fillers.extend(mk_y_half(sb, eh))
        attention_pass(3, 1, after_post0=y_after_stopA)
        # the deferred final norm_post must be emitted while the acc
        # pools are still open (their banks are reused by the tail pool)
        run_deferred()
        # tail: attention PSUM pools close; the remaining W_o blocks
        # drain through a deep PSUM pool so the matmul->copy->DMA
        # chains pipeline instead of serializing on one bank
        attctx.close()
        tail_ps = ctx.enter_context(
            tc.tile_pool(name="tail_ps", bufs=6, space="PSUM")
        )
        ycfg["pool"] = tail_ps
        ycfg["tail"] = True
        for sb in range(12, NSB):
            for eh in range(2):
                fillers.extend(mk_y_half(sb, eh))
        flush_fillers()

        if debug:
            nc.sync.dma_start(dbg["dkT"], kT[:])
            nc.sync.dma_start(dbg["dqz"], qz[:])
            nc.sync.dma_start(dbg["dvT"], vT[:])
            nc.sync.dma_start(dbg["dvn"], vn[:])
            nc.sync.dma_start(dbg["donrm"], onrm[:])

    nc.compile()
    return nc


def get_program():
    global _PROG
    if _PROG is None:
        _PROG = _build_program()
    return _PROG


def make_in_maps(x, W_q, W_k, W_v, W_o):
    perm = _perm64()
    idx_local = (np.arange(DC) // 64) * 64 + perm[np.arange(DC) % 64]
    ang, sgn = _cos_sin_tiles()
    cos_np = np.cos(ang).astype(BF16)
    sin_np = (sgn * np.sin(ang)).astype(BF16)
    # scores tile is (t, q): additive causal mask, 0 where t <= q
    # (keep), MASK_VAL where t > q (exp -> 0)
    cmask_np = np.where(np.triu(np.ones((P, P))) > 0, 0.0,
                        MASK_VAL).astype(BF16)

    def pair8(a):
        # [E, M] -> [128, 4, 2, M] fp8 DoubleRow pair layout
        e, m = a.shape
        return np.ascontiguousarray(
            np.clip(a, -448, 448).reshape(4, 2, P, m).transpose(2, 0, 1, 3)
        ).astype(F8)

    in_maps = []
    for c in range(NCORES):
        b, hg = c // 4, c % 4
        base = hg * DC
        xt = x[b].T
        in_maps.append(
            dict(
                xbt=np.ascontiguousarray(xt.astype(BF16)),
                xbt8=pair8(xt),
                wq8=pair8(W_q[:, base + idx_local] * QK_SCALE),
                wk8=pair8(W_k[:, base + idx_local] * QK_SCALE),
                wv=np.ascontiguousarray(W_v[:, base:base + DC].astype(BF16)),
                wo=np.ascontiguousarray(W_o[base:base + DC, :].astype(BF16)),
                cosr=cos_np,
                sinr=sin_np,
                cmask=cmask_np,
            )
        )
    return in_maps


def kernel(x, W_q, W_k, W_v, W_o, _trace=False, _trace_cores=None):
    from concourse.bass_utils import run_bass_kernel_spmd

    x = np.asarray(x, dtype=np.float32)
    W_q = np.asarray(W_q, dtype=np.float32)
    W_k = np.asarray(W_k, dtype=np.float32)
    W_v = np.asarray(W_v, dtype=np.float32)
    W_o = np.asarray(W_o, dtype=np.float32)

    nc = get_program()
    in_maps = make_in_maps(x, W_q, W_k, W_v, W_o)
    res = run_bass_kernel_spmd(
        nc,
        in_maps,
        list(range(NCORES)),
        trace=_trace,
        trace_cores=_trace_cores,
    )
    y = np.zeros((B, S, E), np.float32)
    for c in range(NCORES):
        y[c // 4] += np.asarray(res.results[c]["y"], dtype=np.float32)
    if _trace:
        return y, res
    return y
